# revision 1
# baseline (speedup 1.0000x reference)
"""Trainium2 Bass kernel for nn_MultiHeadAttention_65773129171319.

Complex-valued multi-head attention:
  attn = softmax(|Qc Kc^H| / sqrt(2 dk)) ; out = (attn @ Vr) Wo, (attn @ Vp) Wo

Sharding: 8 cores = 2 (batch) x 4 (head-groups of 2 heads).  Each core
computes its batch's full sequence for its 2 heads; the out-projection
partial sums (over head groups) are reduced on the host.

Device algorithm (per core, all matmuls bf16, fp32 PSUM accumulation):
  - inputs arrive pre-transposed on host: X^T [D, S] per tensor, bf16
  - Q/K projections produce "stacked" transposed tiles per head:
      qc[h]  = [Qr_h^T ; Qp_h^T]        [128, S]
      kcr[h] = [Kr_h^T ; -Kp_h^T]       [128, S]
      kcp[h] = [Kp_h^T ;  Kr_h^T]       [128, S]
    so that the real/phase score matrices come out of single
    128-contraction matmuls, TRANSPOSED [sk, sq]:
      sT_r[sk,sq] = sum_c kcr[c,sk] qc[c,sq],  sT_p likewise with kcp.
  - u = sT_r^2 + sT_p^2: ACT Square(ps_r) then custom fused DVE op
    SQADD (u = ps_p^2 + u), pipelining ACT against DVE; m = sqrt(u) (ACT),
    attn = exp(m/SCALE) (ACT, bf16 out).  sqrt/exp batch per strip across
    both heads to minimise ACT table-set switches.  Transposed scores let
    attn feed the AV matmul directly as the moving operand.
  - softmax denominators: ones-stationary matmul -> rowsums on partition 0,
    reciprocal (custom DVE approx), GPSIMD partition-broadcast, applied
    while copying the AV output out of PSUM.
  - AV output is stacked per head into xr2hT [128, S] (head h writes PSUM
    partitions h*64..), so the out-projection is one 128-contraction matmul.
"""

import os
import sys

import numpy as np

try:
    import concourse.bass as bass
except ImportError:  # pragma: no cover
    sys.path.insert(0, "/opt/trn_rl_repo")
    import concourse.bass as bass

import ml_dtypes
import concourse.mybir as mybir
import concourse.tile as tile
from concourse import bacc
from concourse.bass_utils import run_bass_kernel_spmd

B, S, D, H = 2, 2048, 512, 8
DK = D // H  # 64
SCALE = float((2 * DK) ** 0.5)
P = 128
N_CORES = 8
HG = 4            # head groups (2 heads each)
DT = D // P       # 4 d-tiles for projection contraction
SKT = S // P      # 16 sk tiles
NSTRIP = 4        # sq strips of 512
STRIP = S // NSTRIP  # 512

F32 = mybir.dt.float32
BF16 = mybir.dt.bfloat16
BFNP = ml_dtypes.bfloat16

AF = mybir.ActivationFunctionType


def register_custom_ops():
    """Register fused DVE ops (runtime extension of dve_ops.OPS)."""
    import concourse.dve_ops as dve_ops
    from concourse.dve_ops import DveOp
    from concourse.dve_spec import Spec, Src0, Src1, sq, lower, _has_src1
    from concourse.dve_uop import DveOpSpec

    existing = {op.name: op for op in dve_ops.OPS}

    def mk(name, spec):
        if name in existing:
            return existing[name]
        row = max(dve_ops._SUB_OPCODE_FOR_NAME.values()) + 1
        assert row < 0x20, "no free DVE opcode rows"
        dve_ops._SUB_OPCODE_FOR_NAME[name] = row
        shas = {}
        for ver in ("v3", "v4"):
            s = DveOpSpec(name=name, opcode=row, uops=lower(spec, ver=ver),
                          rd1_en=_has_src1(spec))
            shas[ver] = s.sha(ver)
        op = DveOp(name, spec, subdim=False, uops_sha=shas)
        dve_ops.OPS.append(op)
        return op

    sq1 = mk("SQ1_ANT", Spec(
        body=sq(Src0),
        reference=lambda in0, in1, s0, s1, imm2: in0.astype(np.float32) ** 2))
    sqadd = mk("SQADD_ANT", Spec(
        body=sq(Src0) + Src1,
        reference=lambda in0, in1, s0, s1, imm2:
            in0.astype(np.float32) ** 2 + in1.astype(np.float32)))
    return sq1, sqadd


SQ1, SQADD = register_custom_ops()


def build(n_iter: int = 1, variant: frozenset = frozenset()):
    """Build (and bacc-compile) the per-core SPMD program."""
    nc = bacc.Bacc("TRN2", target_bir_lowering=False, debug=False,
                   num_devices=N_CORES)

    dr = {}
    for name in ("xqr", "xqp", "xkr", "xkp", "xvr", "xvp"):
        dr[name] = nc.dram_tensor(name, [D, S], BF16, kind="ExternalInput")
    for name in ("wq", "wk", "wv"):
        dr[name] = nc.dram_tensor(name, [D, 2 * DK], BF16, kind="ExternalInput")
    dr["wo"] = nc.dram_tensor("wo", [2 * DK, D], BF16, kind="ExternalInput")
    dr["o_r"] = nc.dram_tensor("o_r", [S, D], F32, kind="ExternalOutput")
    dr["o_p"] = nc.dram_tensor("o_p", [S, D], F32, kind="ExternalOutput")

    with tile.TileContext(nc) as tc:
        _emit(tc, dr, n_iter, variant)
    nc.compile()
    return nc


def _emit(tc, dr, n_iter, variant=frozenset()):
    from contextlib import ExitStack

    ctx = ExitStack()
    with ctx:
        pools = dict(
            singles=ctx.enter_context(tc.tile_pool(name="singles", bufs=1)),
            xpool=ctx.enter_context(tc.tile_pool(name="xp", bufs=3)),
            upool=ctx.enter_context(tc.tile_pool(name="up", bufs=3)),
            apool=ctx.enter_context(tc.tile_pool(name="ap", bufs=3)),
            tpool=ctx.enter_context(tc.tile_pool(name="tp", bufs=3)),
            opool=ctx.enter_context(tc.tile_pool(name="op", bufs=3)),
            psA=ctx.enter_context(tc.tile_pool(name="psA", bufs=3, space="PSUM")),
            psAV=ctx.enter_context(tc.tile_pool(name="psAV", bufs=3, space="PSUM")),
            psO=ctx.enter_context(tc.tile_pool(name="psO", bufs=2, space="PSUM")),
        )
        if n_iter > 1:
            with tc.For_i(0, n_iter, 1):
                _body(tc, dr, variant, **pools)
        else:
            _body(tc, dr, variant, **pools)


def _body(tc, dr, variant, singles, xpool, upool, apool, tpool, opool, psA, psAV, psO):
    nc = tc.nc

    # ---- weights to SBUF -------------------------------------------------
    wsb = {}
    for name in ("wq", "wk", "wv"):
        t = singles.tile([P, DT, 2 * DK], BF16, tag=f"w_{name}", name=f"w_{name}")
        nc.sync.dma_start(out=t[:], in_=dr[name].rearrange("(dt p) m -> p dt m", p=P))
        wsb[name] = t
    wkn = singles.tile([P, DT, 2 * DK], BF16, tag="w_wkn", name="w_wkn")
    nc.scalar.mul(out=wkn[:], in_=wsb["wk"][:], mul=-1.0)
    wo = singles.tile([P, D], BF16, tag="w_wo", name="w_wo")
    nc.sync.dma_start(out=wo[:], in_=dr["wo"][:])
    ones = singles.tile([P, 1], BF16, tag="ones", name="ones")
    nc.vector.memset(ones[:], 1.0)

    # ---- persistent SBUF tensors ----------------------------------------
    qc = [singles.tile([P, S], BF16, tag=f"qc{h}", name=f"qc{h}") for h in range(2)]
    kcr = [singles.tile([P, S], BF16, tag=f"kcr{h}", name=f"kcr{h}") for h in range(2)]
    kcp = [singles.tile([P, S], BF16, tag=f"kcp{h}", name=f"kcp{h}") for h in range(2)]
    vtr = [singles.tile([P, SKT, DK], BF16, tag=f"vtr{h}", name=f"vtr{h}") for h in range(2)]
    vtp = [singles.tile([P, SKT, DK], BF16, tag=f"vtp{h}", name=f"vtp{h}") for h in range(2)]
    xr2hT = singles.tile([P, S], BF16, tag="xr2hT", name="xr2hT")
    xp2hT = singles.tile([P, S], BF16, tag="xp2hT", name="xp2hT")

    def _xdma(out, in_):
        if "nodma" not in variant:
            nc.sync.dma_start(out=out, in_=in_)

    # ---- K projection ----------------------------------------------------
    for s in range(NSTRIP):
        ssl = slice(s * STRIP, (s + 1) * STRIP)
        xtr = xpool.tile([P, DT, STRIP], BF16, tag="xs", name="xs")
        _xdma(xtr[:], dr["xkr"].rearrange("(dt p) s -> p dt s", p=P)[:, :, ssl])
        xtp = xpool.tile([P, DT, STRIP], BF16, tag="xs", name="xs")
        _xdma(xtp[:], dr["xkp"].rearrange("(dt p) s -> p dt s", p=P)[:, :, ssl])
        for h in range(2):
            if "noproj" in variant:
                break
            hsl = slice(h * DK, (h + 1) * DK)
            ps_kcr = psA.tile([P, STRIP], F32, tag="psA", name="psA")
            ps_kcp = psA.tile([P, STRIP], F32, tag="psA", name="psA")
            for dt in range(DT):
                st = (dt == 0)
                sp = (dt == DT - 1)
                nc.tensor.matmul(ps_kcr[0:DK, :], wsb["wk"][:, dt, hsl],
                                 xtr[:, dt, :], start=st, stop=sp)
                nc.tensor.matmul(ps_kcr[DK:P, :], wkn[:, dt, hsl],
                                 xtp[:, dt, :], start=st, stop=sp)
                nc.tensor.matmul(ps_kcp[0:DK, :], wsb["wk"][:, dt, hsl],
                                 xtp[:, dt, :], start=st, stop=sp)
                nc.tensor.matmul(ps_kcp[DK:P, :], wsb["wk"][:, dt, hsl],
                                 xtr[:, dt, :], start=st, stop=sp)
            nc.vector.tensor_copy(kcr[h][:, ssl], ps_kcr[:])
            nc.vector.tensor_copy(kcp[h][:, ssl], ps_kcp[:])

    # ---- Q projection ----------------------------------------------------
    for s in range(NSTRIP):
        ssl = slice(s * STRIP, (s + 1) * STRIP)
        xtr = xpool.tile([P, DT, STRIP], BF16, tag="xs", name="xs")
        _xdma(xtr[:], dr["xqr"].rearrange("(dt p) s -> p dt s", p=P)[:, :, ssl])
        xtp = xpool.tile([P, DT, STRIP], BF16, tag="xs", name="xs")
        _xdma(xtp[:], dr["xqp"].rearrange("(dt p) s -> p dt s", p=P)[:, :, ssl])
        for h in range(2):
            if "noproj" in variant:
                break
            hsl = slice(h * DK, (h + 1) * DK)
            ps_q = psA.tile([P, STRIP], F32, tag="psA", name="psA")
            for dt in range(DT):
                st = (dt == 0)
                sp = (dt == DT - 1)
                nc.tensor.matmul(ps_q[0:DK, :], wsb["wq"][:, dt, hsl],
                                 xtr[:, dt, :], start=st, stop=sp)
                nc.tensor.matmul(ps_q[DK:P, :], wsb["wq"][:, dt, hsl],
                                 xtp[:, dt, :], start=st, stop=sp)
            nc.vector.tensor_copy(qc[h][:, ssl], ps_q[:])

    # ---- V projection ----------------------------------------------------
    for kind, src, dst in (("r", "xvr", vtr), ("p", "xvp", vtp)):
        for s in range(NSTRIP):
            xt = xpool.tile([P, DT, STRIP], BF16, tag="xs", name="xs")
            _xdma(xt[:], dr[src].rearrange("(dt p) s -> p dt s", p=P)[
                :, :, s * STRIP:(s + 1) * STRIP])
            for tt in range(STRIP // P):
                if "noproj" in variant:
                    break
                t = s * (STRIP // P) + tt
                ps_v = psAV.tile([P, STRIP], F32, tag="av", name="av")
                for dt in range(DT):
                    nc.tensor.matmul(ps_v[:, 0:2 * DK],
                                     xt[:, dt, tt * P:(tt + 1) * P],
                                     wsb["wv"][:, dt, :],
                                     start=(dt == 0), stop=(dt == DT - 1))
                for h in range(2):
                    nc.vector.tensor_copy(dst[h][:, t, 0:DK],
                                          ps_v[:, h * DK:(h + 1) * DK])

    # ---- attention -------------------------------------------------------
    # loop: strip outer, head inner; both heads' u computed before the
    # sqrt/exp pair so ACT table switches batch (2 per strip).
    inv_scale = 1.0 / SCALE
    for s in range(NSTRIP):
        ssl = slice(s * STRIP, (s + 1) * STRIP)
        us = []
        for h in range(2):
            u = upool.tile([P, SKT, STRIP], BF16, tag="u", name="u")
            us.append(u)
            for t in range(SKT):
                tsl = slice(t * P, (t + 1) * P)
                ps_r = psA.tile([P, STRIP], F32, tag="psA", name="psA")
                if "noscores" not in variant:
                    nc.tensor.matmul(ps_r[:], kcr[h][:, tsl], qc[h][:, ssl],
                                     start=True, stop=True)
                ps_p = psA.tile([P, STRIP], F32, tag="psA", name="psA")
                if "noscores" not in variant:
                    nc.tensor.matmul(ps_p[:], kcp[h][:, tsl], qc[h][:, ssl],
                                     start=True, stop=True)
                if "nosq" in variant:
                    if t == 0:
                        nc.vector.memset(u[:], 0.25)
                    continue
                if t % 3 != 0:
                    nc.scalar.square(u[:, t, :], ps_r[:])
                else:
                    nc.vector._custom_dve(SQ1, out=u[:, t, :], in0=ps_r[:])
                nc.vector._custom_dve(SQADD, out=u[:, t, :], in0=ps_p[:],
                                      in1=u[:, t, :])
        if "nosqrtexp" not in variant:
            for h in range(2):
                nc.scalar.activation(us[h][:], us[h][:], AF.Sqrt)
        attns = []
        for h in range(2):
            attn = apool.tile([P, SKT, STRIP], BF16, tag="attn", name="attn")
            attns.append(attn)
            if "nosqrtexp" in variant:
                nc.vector.tensor_copy(attn[:], us[h][:])
            else:
                nc.scalar.activation(attn[:], us[h][:], AF.Exp, scale=inv_scale)
        for h in range(2):
            attn = attns[h]
            hps = slice(h * DK, (h + 1) * DK)
            # rowsums -> partition 0 (ones stationary, M=1)
            ps_rs = psAV.tile([P, STRIP], F32, tag="av", name="av")
            if "norowsum" not in variant:
                for t in range(SKT):
                    nc.tensor.matmul(ps_rs[0:1, :], ones[:], attn[:, t, :],
                                     start=(t == 0), stop=(t == SKT - 1))
            rrec = tpool.tile([1, STRIP], F32, tag="rrec", name="rrec")
            if "norecip" in variant:
                nc.vector.memset(rrec[:], 1.0)
            else:
                nc.vector.reciprocal_approx_fast(rrec[:], ps_rs[0:1, :])
            rb = tpool.tile([P, STRIP], F32, tag="rb", name="rb")
            if "nobcast" in variant:
                nc.vector.memset(rb[:], 1.0)
            else:
                nc.gpsimd.partition_broadcast(rb[:], rrec[:])
            # AV: head h lands on PSUM partitions h*64..h*64+63
            ps_avr = psAV.tile([P, STRIP], F32, tag="av", name="av")
            if "noav" not in variant:
                for t in range(SKT):
                    nc.tensor.matmul(ps_avr[hps, :], vtr[h][:, t, :],
                                     attn[:, t, :], start=(t == 0),
                                     stop=(t == SKT - 1))
            ps_avp = psAV.tile([P, STRIP], F32, tag="av", name="av")
            if "noav" not in variant:
                for t in range(SKT):
                    nc.tensor.matmul(ps_avp[hps, :], vtp[h][:, t, :],
                                     attn[:, t, :], start=(t == 0),
                                     stop=(t == SKT - 1))
            nc.vector.tensor_mul(xr2hT[hps, ssl], ps_avr[hps, :], rb[hps, :])
            nc.vector.tensor_mul(xp2hT[hps, ssl], ps_avp[hps, :], rb[hps, :])

    # ---- out projection --------------------------------------------------
    for kind, xT, out in (("r", xr2hT, dr["o_r"]), ("p", xp2hT, dr["o_p"])):
        if "noout" in variant:
            break
        for q in range(S // P):
            qsl = slice(q * P, (q + 1) * P)
            ps_o = psO.tile([P, D], F32, tag="o", name="o")
            nc.tensor.matmul(ps_o[:], xT[:, qsl], wo[:], start=True, stop=True)
            osb = opool.tile([P, D], F32, tag="osb", name="osb")
            nc.vector.tensor_copy(osb[:], ps_o[:])
            nc.sync.dma_start(out=out[qsl, :], in_=osb[:])


# ---------------------------------------------------------------------------
_CACHE = {}


def _get_nc(n_iter=1, variant=frozenset()):
    key = (n_iter, variant)
    if key not in _CACHE:
        _CACHE[key] = build(n_iter, variant)
    return _CACHE[key]


def make_in_maps(q_real, k_real, v_real, q_phase, k_phase, v_phase,
                 w_q, w_k, w_v, w_o):
    """Host-side shard + layout prep: per-core input dicts."""
    xt = {}
    for b in range(B):
        xt[("xqr", b)] = np.ascontiguousarray(q_real[b].T).astype(BFNP)
        xt[("xqp", b)] = np.ascontiguousarray(q_phase[b].T).astype(BFNP)
        xt[("xkr", b)] = np.ascontiguousarray(k_real[b].T).astype(BFNP)
        xt[("xkp", b)] = np.ascontiguousarray(k_phase[b].T).astype(BFNP)
        xt[("xvr", b)] = np.ascontiguousarray(v_real[b].T).astype(BFNP)
        xt[("xvp", b)] = np.ascontiguousarray(v_phase[b].T).astype(BFNP)
    wq16, wk16, wv16, wo16 = (w.astype(BFNP) for w in (w_q, w_k, w_v, w_o))
    in_maps = []
    for core in range(N_CORES):
        b, hg = divmod(core, HG)
        csl = slice(hg * 2 * DK, (hg + 1) * 2 * DK)
        in_maps.append({
            "xqr": xt[("xqr", b)], "xqp": xt[("xqp", b)],
            "xkr": xt[("xkr", b)], "xkp": xt[("xkp", b)],
            "xvr": xt[("xvr", b)], "xvp": xt[("xvp", b)],
            "wq": np.ascontiguousarray(wq16[:, csl]),
            "wk": np.ascontiguousarray(wk16[:, csl]),
            "wv": np.ascontiguousarray(wv16[:, csl]),
            "wo": np.ascontiguousarray(wo16[csl, :]),
        })
    return in_maps


def gather_outputs(results):
    out_r = np.zeros((B, S, D), np.float32)
    out_p = np.zeros((B, S, D), np.float32)
    for core in range(N_CORES):
        b = core // HG
        out_r[b] += results[core]["o_r"]
        out_p[b] += results[core]["o_p"]
    return out_r, out_p


def _numpy_fallback(q_real, k_real, v_real, q_phase, k_phase, v_phase,
                    w_q, w_k, w_v, w_o, mask):
    def heads(x, w):
        y = x @ w
        return y.reshape(B, -1, H, DK).transpose(0, 2, 1, 3)
    qr, kr, vr = heads(q_real, w_q), heads(k_real, w_k), heads(v_real, w_v)
    qp, kp, vp = heads(q_phase, w_q), heads(k_phase, w_k), heads(v_phase, w_v)
    ar = np.einsum('bhqd,bhkd->bhqk', qr, kr) - np.einsum('bhqd,bhkd->bhqk', qp, kp)
    ap = np.einsum('bhqd,bhkd->bhqk', qr, kp) + np.einsum('bhqd,bhkd->bhqk', qp, kr)
    a = np.sqrt(ar * ar + ap * ap) / SCALE
    a = np.where(mask[:, None, :, :] == 0, np.float32(-1e9), a)
    a = a - a.max(axis=-1, keepdims=True)
    e = np.exp(a)
    a = e / e.sum(axis=-1, keepdims=True)
    xr = np.einsum('bhqk,bhkd->bhqd', a, vr).transpose(0, 2, 1, 3).reshape(B, -1, D)
    xp = np.einsum('bhqk,bhkd->bhqd', a, vp).transpose(0, 2, 1, 3).reshape(B, -1, D)
    return (xr @ w_o).astype(np.float32), (xp @ w_o).astype(np.float32)


def kernel(q_real, k_real, v_real, q_phase, k_phase, v_phase,
           w_q, w_k, w_v, w_o, mask):
    args = [np.asarray(a, np.float32) for a in
            (q_real, k_real, v_real, q_phase, k_phase, v_phase,
             w_q, w_k, w_v, w_o)]
    mask = np.asarray(mask)
    if not np.all(mask != 0):
        return _numpy_fallback(*args, mask)
    nc = _get_nc(1)
    in_maps = make_in_maps(*args)
    res = run_bass_kernel_spmd(nc, in_maps, core_ids=list(range(N_CORES)))
    return gather_outputs(res.results)



# revision 8
# speedup vs baseline: 1.0832x; 1.0832x over previous
"""Trainium2 Bass kernel for nn_MultiHeadAttention_65773129171319.

Complex-valued multi-head attention:
  attn = softmax(|Qc Kc^H| / sqrt(2 dk)) ; out = (attn @ Vr) Wo, (attn @ Vp) Wo

Sharding: 8 cores = 2 (batch) x 4 (head-groups of 2 heads).  Each core
computes its batch's full sequence for its 2 heads; the out-projection
partial sums (over head groups) are reduced on the host.

V2 design (all fp16 on device, fp32 PSUM):
  - Packed 2-head K/Q projections: one M=128 matmul per (plane, d-tile)
    computes both heads at once; plane pairs land in a single 2-bank PSUM
    tile, one ACT copy stages them to SBUF, and the GPSIMD (pool) engine
    repacks them into per-head stacked score operands:
      qc[0]=[Qr_h0;Qp_h0]  qc[1]=[Qp_h1;Qr_h1]
      kcr[0]=[Kr_h0;-Kp_h0] kcr[1]=[-Kp_h1;Kr_h1]
      kcp[0]=[Kp_h0;Kr_h0]  kcp[1]=[Kr_h1;Kp_h1]
    (The phase-plane matmuls use head-swapped weight copies so most of the
    repack copies are partition-offset-free.)
  - Scores per (strip, head): sT_r = kcr^T qc, sT_p = kcp^T qc as single
    C=128 matmuls producing transposed [sk, sq] tiles, two t-tiles per
    2-bank PSUM tile.  u = sT_r^2 + sT_p^2 via ACT Square + DVE SQADD.
  - m = sqrt(u) on ACT (the only table function -> zero table switches),
    attn = exp(m/SCALE) on DVE via custom EXP8 op:
      exp(m/s) ~= (((a*m + b)*m + c)^2)^2)^2   (max rel err ~5e-4)
  - Rowsums via ones-stationary matmul; reciprocal + partition broadcast;
    applied to the AV output (small side).
  - AV packed per head: stationary [vr_h|vp_h] (head1: [vp|vr]) gives
    M=128 AV matmuls; results scatter offset-free into xrT=[h0r;h1r] and
    xpT=[h1p;h0p]; out-projection uses wo_A (natural rows) for o_r and
    wo_B (head-swapped rows) for o_p.
  - Emission is software-pipelined over units w=(strip, head) in slots:
    scores(w) | sqrt+exp(w-1) | rowsum/AV/normalize(w-2), so PE, ACT and
    DVE always have a slot of ready work.
"""

import os
import sys

import numpy as np

try:
    import concourse.bass as bass
except ImportError:  # pragma: no cover
    sys.path.insert(0, "/opt/trn_rl_repo")
    import concourse.bass as bass

import concourse.mybir as mybir
import concourse.tile as tile
from concourse import bacc
from concourse.bass_utils import run_bass_kernel_spmd

B, S, D, H = 2, 2048, 512, 8
DK = D // H  # 64
SCALE = float((2 * DK) ** 0.5)  # sqrt(128)
P = 128
N_CORES = 8
HG = 4            # head groups (2 heads each)
DT = D // P       # 4 d-tiles for projection contraction
SKT = S // P      # 16 sk tiles
NSTRIP = 4        # sq strips of 512
STRIP = S // NSTRIP  # 512
NPAIR = SKT // 2  # t-pairs per (strip, head)

F32 = mybir.dt.float32
F16 = mybir.dt.float16
F16NP = np.float16

AF = mybir.ActivationFunctionType

# EXP8 poly coefficients: exp(m/SCALE) ~= (((EA*m+EB)*m+EC)^2^2)^2,
# fit on m in [0, 18] (empirical max |z| ~ 15.8); max rel err 5.3e-4.
EA = 6.734965764779986e-05
EB = 0.011003405951248851
EC = 1.0000654804195346


def register_custom_ops():
    """Register fused DVE ops (runtime extension of dve_ops.OPS)."""
    import concourse.dve_ops as dve_ops
    from concourse.dve_ops import DveOp
    from concourse.dve_spec import Spec, Src0, Src1, C0, C1, C2, sq, lower, _has_src1
    from concourse.dve_uop import DveOpSpec

    existing = {op.name: op for op in dve_ops.OPS}

    def mk(name, spec):
        if name in existing:
            return existing[name]
        row = max(dve_ops._SUB_OPCODE_FOR_NAME.values()) + 1
        assert row < 0x20, "no free DVE opcode rows"
        dve_ops._SUB_OPCODE_FOR_NAME[name] = row
        shas = {}
        for ver in ("v3", "v4"):
            s = DveOpSpec(name=name, opcode=row, uops=lower(spec, ver=ver),
                          rd1_en=_has_src1(spec))
            shas[ver] = s.sha(ver)
        op = DveOp(name, spec, subdim=False, uops_sha=shas)
        dve_ops.OPS.append(op)
        return op

    sqadd = mk("SQADD_ANT", Spec(
        body=sq(Src0) + Src1,
        reference=lambda in0, in1, s0, s1, imm2:
            in0.astype(np.float32) ** 2 + in1.astype(np.float32)))
    def _exp8_ref(in0, in1, s0, s1, imm2):
        p = (s0 * in0.astype(np.float32) + s1) * in0.astype(np.float32) + imm2
        return ((p ** 2) ** 2) ** 2

    exp8 = mk("EXP8_ANT", Spec(
        body=sq(sq(sq((Src0 * C0 + C1) * Src0 + C2))),
        reference=_exp8_ref))
    return sqadd, exp8


SQADD, EXP8 = register_custom_ops()


def build(n_iter: int = 1, variant: frozenset = frozenset()):
    """Build (and bacc-compile) the per-core SPMD program."""
    nc = bacc.Bacc("TRN2", target_bir_lowering=False, debug=False,
                   num_devices=N_CORES)

    dr = {}
    for name in ("xqr", "xqp", "xkr", "xkp", "xvr", "xvp"):
        dr[name] = nc.dram_tensor(name, [D, S], F16, kind="ExternalInput")
    for name in ("wq_n", "wq_s", "wk_n", "wk_s", "wv_n", "wv_s"):
        dr[name] = nc.dram_tensor(name, [D, 2 * DK], F16, kind="ExternalInput")
    dr["wo_a"] = nc.dram_tensor("wo_a", [2 * DK, D], F16, kind="ExternalInput")
    dr["wo_b"] = nc.dram_tensor("wo_b", [2 * DK, D], F16, kind="ExternalInput")
    dr["o_r"] = nc.dram_tensor("o_r", [S, D], F16, kind="ExternalOutput")
    dr["o_p"] = nc.dram_tensor("o_p", [S, D], F16, kind="ExternalOutput")

    with tile.TileContext(nc) as tc:
        _emit(tc, dr, n_iter, variant)
    nc.compile()
    return nc


def _emit(tc, dr, n_iter, variant=frozenset()):
    from contextlib import ExitStack

    ctx = ExitStack()
    with ctx:
        pools = dict(
            singles=ctx.enter_context(tc.tile_pool(name="singles", bufs=1)),
            xpool=ctx.enter_context(tc.tile_pool(name="xp", bufs=3)),
            ppool=ctx.enter_context(tc.tile_pool(name="pp", bufs=2)),
            upool=ctx.enter_context(tc.tile_pool(name="up", bufs=4)),
            rbpool=ctx.enter_context(tc.tile_pool(name="rb", bufs=2)),
            rrpool=ctx.enter_context(tc.tile_pool(name="rr", bufs=2)),
            opool=ctx.enter_context(tc.tile_pool(name="op", bufs=3)),
            psA=ctx.enter_context(tc.tile_pool(name="psA", bufs=2, space="PSUM")),
            psRS=ctx.enter_context(tc.tile_pool(name="psRS", bufs=1, space="PSUM")),
            psAV=ctx.enter_context(tc.tile_pool(name="psAV", bufs=2, space="PSUM")),
            psO=ctx.enter_context(tc.tile_pool(name="psO", bufs=1, space="PSUM")),
        )
        if n_iter > 1:
            with tc.For_i(0, n_iter, 1):
                _body(tc, dr, variant, **pools)
        else:
            _body(tc, dr, variant, **pools)


def _body(tc, dr, variant, singles, xpool, ppool, upool, rbpool, rrpool,
          opool, psA, psRS, psAV, psO):
    nc = tc.nc
    V = lambda name: name in variant

    # ---- weights to SBUF -------------------------------------------------
    wsb = {}
    for name in ("wq_n", "wq_s", "wk_n", "wk_s", "wv_n", "wv_s"):
        t = singles.tile([P, DT, 2 * DK], F16, tag=f"w_{name}", name=f"w_{name}")
        nc.sync.dma_start(out=t[:], in_=dr[name].rearrange("(dt p) m -> p dt m", p=P))
        wsb[name] = t
    wo_a = singles.tile([P, D], F16, tag="w_wo_a", name="w_wo_a")
    nc.sync.dma_start(out=wo_a[:], in_=dr["wo_a"][:])
    wo_b = singles.tile([P, D], F16, tag="w_wo_b", name="w_wo_b")
    nc.sync.dma_start(out=wo_b[:], in_=dr["wo_b"][:])
    ones = singles.tile([P, 1], F16, tag="ones", name="ones")
    nc.vector.memset(ones[:], 1.0)

    # ---- persistent SBUF tensors ----------------------------------------
    kcr = [singles.tile([P, S], F16, tag=f"kcr{h}", name=f"kcr{h}") for h in range(2)]
    kcp = [singles.tile([P, S], F16, tag=f"kcp{h}", name=f"kcp{h}") for h in range(2)]
    qc = [singles.tile([P, S], F16, tag=f"qc{h}", name=f"qc{h}") for h in range(2)]
    v2 = [singles.tile([P, SKT, P], F16, tag=f"v2_{h}", name=f"v2_{h}") for h in range(2)]
    xrT = singles.tile([P, S], F16, tag="xrT", name="xrT")
    xpT = singles.tile([P, S], F16, tag="xpT", name="xpT")

    def _xdma(out, in_):
        if not V("nodma"):
            nc.sync.dma_start(out=out, in_=in_)

    def _xs(name, ssl):
        t = xpool.tile([P, DT, STRIP], F16, tag="xs", name="xs")
        _xdma(t[:], dr[name].rearrange("(dt p) s -> p dt s", p=P)[:, :, ssl])
        return t

    # ---- K projection + repack ------------------------------------------
    def k_proj(s):
        ssl = slice(s * STRIP, (s + 1) * STRIP)
        xtr = _xs("xkr", ssl)
        xtp = _xs("xkp", ssl)
        psK = psA.tile([P, 2, STRIP], F32, tag="psA", name="psA")
        if not V("noproj"):
            for dt in range(DT):
                st, sp = (dt == 0), (dt == DT - 1)
                nc.tensor.matmul(psK[:, 0, :], wsb["wk_n"][:, dt, :],
                                 xtr[:, dt, :], start=st, stop=sp)
                nc.tensor.matmul(psK[:, 1, :], wsb["wk_s"][:, dt, :],
                                 xtp[:, dt, :], start=st, stop=sp)
        pl = ppool.tile([P, 2, STRIP], F16, tag="pl", name="pl")
        nc.scalar.copy(pl[:], psK[:])
        lo, hi = slice(0, DK), slice(DK, P)
        gp = nc.gpsimd
        # plane 0 = [Kr_h0; Kr_h1], plane 1 = [Kp_h1; Kp_h0] (partition halves)
        gp.tensor_copy(kcr[0][lo, ssl], pl[lo, 0, :])
        gp.tensor_scalar_mul(kcr[0][hi, ssl], pl[hi, 1, :], -1.0)
        gp.tensor_scalar_mul(kcr[1][lo, ssl], pl[lo, 1, :], -1.0)
        gp.tensor_copy(kcr[1][hi, ssl], pl[hi, 0, :])
        gp.tensor_copy(kcp[0][lo, ssl], pl[hi, 1, :])
        gp.tensor_copy(kcp[0][hi, ssl], pl[lo, 0, :])
        gp.tensor_copy(kcp[1][lo, ssl], pl[hi, 0, :])
        gp.tensor_copy(kcp[1][hi, ssl], pl[lo, 1, :])

    # ---- Q projection + repack ------------------------------------------
    def q_proj(s):
        ssl = slice(s * STRIP, (s + 1) * STRIP)
        xtr = _xs("xqr", ssl)
        xtp = _xs("xqp", ssl)
        psQ = psA.tile([P, 2, STRIP], F32, tag="psA", name="psA")
        if not V("noproj"):
            for dt in range(DT):
                st, sp = (dt == 0), (dt == DT - 1)
                nc.tensor.matmul(psQ[:, 0, :], wsb["wq_n"][:, dt, :],
                                 xtr[:, dt, :], start=st, stop=sp)
                nc.tensor.matmul(psQ[:, 1, :], wsb["wq_s"][:, dt, :],
                                 xtp[:, dt, :], start=st, stop=sp)
        pl = ppool.tile([P, 2, STRIP], F16, tag="pl", name="pl")
        nc.scalar.copy(pl[:], psQ[:])
        lo, hi = slice(0, DK), slice(DK, P)
        gp = nc.gpsimd
        # plane 0 = [Qr_h0; Qr_h1], plane 1 = [Qp_h1; Qp_h0]
        gp.tensor_copy(qc[0][lo, ssl], pl[lo, 0, :])
        gp.tensor_copy(qc[0][hi, ssl], pl[hi, 1, :])
        gp.tensor_copy(qc[1][lo, ssl], pl[lo, 1, :])
        gp.tensor_copy(qc[1][hi, ssl], pl[hi, 0, :])

    # ---- V projection ----------------------------------------------------
    def v_proj(s):
        xvr_t = _xs("xvr", slice(s * STRIP, (s + 1) * STRIP))
        xvp_t = _xs("xvp", slice(s * STRIP, (s + 1) * STRIP))
        for tt in range(STRIP // P):
            t = s * (STRIP // P) + tt
            tpsl = slice(tt * P, (tt + 1) * P)
            psV = psA.tile([P, 2, STRIP], F32, tag="psA", name="psA")
            if not V("noproj"):
                for dt in range(DT):
                    st, sp = (dt == 0), (dt == DT - 1)
                    nc.tensor.matmul(psV[:, 0, 0:2 * DK], xvr_t[:, dt, tpsl],
                                     wsb["wv_n"][:, dt, :], start=st, stop=sp)
                    nc.tensor.matmul(psV[:, 1, 0:2 * DK], xvp_t[:, dt, tpsl],
                                     wsb["wv_s"][:, dt, :], start=st, stop=sp)
            # psV[:, 0, :] = [vr_h0 | vr_h1], psV[:, 1, :] = [vp_h1 | vp_h0]
            nc.scalar.copy(v2[0][:, t, 0:DK], psV[:, 0, 0:DK])
            nc.scalar.copy(v2[0][:, t, DK:2 * DK], psV[:, 1, DK:2 * DK])
            nc.scalar.copy(v2[1][:, t, 0:DK], psV[:, 1, 0:DK])
            nc.scalar.copy(v2[1][:, t, DK:2 * DK], psV[:, 0, DK:2 * DK])

    # ---- attention stages -----------------------------------------------
    units = [(s, h) for s in range(NSTRIP) for h in range(2)]
    u_tiles = {}

    def stage_a(w):
        s, h = w
        ssl = slice(s * STRIP, (s + 1) * STRIP)
        u = upool.tile([P, SKT, STRIP], F16, tag="u", name="u")
        u_tiles[w] = u
        for k in range(NPAIR):
            psr = psA.tile([P, 2, STRIP], F32, tag="psA", name="psA")
            psp = psA.tile([P, 2, STRIP], F32, tag="psA", name="psA")
            for j in range(2):
                t = 2 * k + j
                tsl = slice(t * P, (t + 1) * P)
                if not V("noscores"):
                    nc.tensor.matmul(psr[:, j, :], kcr[h][:, tsl], qc[h][:, ssl],
                                     start=True, stop=True)
                    nc.tensor.matmul(psp[:, j, :], kcp[h][:, tsl], qc[h][:, ssl],
                                     start=True, stop=True)
            usl = u[:, 2 * k:2 * k + 2, :]
            if V("nosq"):
                if k == 0:
                    nc.vector.memset(u[:], 1.0)
                continue
            nc.scalar.square(usl, psr[:])
            nc.vector._custom_dve(SQADD, out=usl, in0=psp[:], in1=usl)

    def stage_bc(w):
        if V("nosqrtexp"):
            return
        u = u_tiles[w]
        half = SKT // 2
        for c in range(2):
            csl = u[:, c * half:(c + 1) * half, :]
            nc.scalar.activation(csl, csl, AF.Sqrt)
        for c in range(2):
            csl = u[:, c * half:(c + 1) * half, :]
            nc.vector._custom_dve(EXP8, out=csl, in0=csl, s0=EA, s1=EB, imm2=EC)

    def stage_def(w):
        s, h = w
        ssl = slice(s * STRIP, (s + 1) * STRIP)
        attn = u_tiles.pop(w)
        ps_rs = psRS.tile([1, STRIP], F32, tag="rs", name="rs")
        if not V("norowsum"):
            for t in range(SKT):
                nc.tensor.matmul(ps_rs[0:1, :], ones[:], attn[:, t, :],
                                 start=(t == 0), stop=(t == SKT - 1))
        rrec = rrpool.tile([1, STRIP], F32, tag="rrec", name="rrec")
        if V("norecip"):
            nc.vector.memset(rrec[:], 1.0)
        else:
            nc.vector.reciprocal_approx_fast(rrec[:], ps_rs[0:1, :])
        rb = rbpool.tile([P, STRIP], F32, tag="rb", name="rb")
        if V("nobcast"):
            nc.vector.memset(rb[:], 1.0)
        else:
            nc.gpsimd.partition_broadcast(rb[:], rrec[:])
        ps_av = psAV.tile([P, STRIP], F32, tag="av", name="av")
        if not V("noav"):
            for t in range(SKT):
                nc.tensor.matmul(ps_av[:], v2[h][:, t, :], attn[:, t, :],
                                 start=(t == 0), stop=(t == SKT - 1))
        lo, hi = slice(0, DK), slice(DK, P)
        if h == 0:
            nc.vector.tensor_mul(xrT[lo, ssl], ps_av[lo, :], rb[lo, :])
            nc.vector.tensor_mul(xpT[hi, ssl], ps_av[hi, :], rb[hi, :])
        else:
            nc.vector.tensor_mul(xpT[lo, ssl], ps_av[lo, :], rb[lo, :])
            nc.vector.tensor_mul(xrT[hi, ssl], ps_av[hi, :], rb[hi, :])

    def stage_g(s):
        if V("noout"):
            return
        for q in range(STRIP // P):
            qsl = slice((s * (STRIP // P) + q) * P, (s * (STRIP // P) + q + 1) * P)
            for xT, wo, out in ((xrT, wo_a, dr["o_r"]), (xpT, wo_b, dr["o_p"])):
                ps_o = psO.tile([P, D], F32, tag="o", name="o")
                nc.tensor.matmul(ps_o[:], xT[:, qsl], wo[:], start=True, stop=True)
                osb = opool.tile([P, D], F16, tag="osb", name="osb")
                nc.scalar.copy(osb[:], ps_o[:])
                _xdma(out[qsl, :], osb[:])

    # ---- emission: projections then software-pipelined attention --------
    for s in range(NSTRIP):
        k_proj(s)
    for s in range(NSTRIP):
        q_proj(s)
    for s in range(NSTRIP):
        v_proj(s)

    n = len(units)
    for slot in range(n + 2):
        if slot < n:
            stage_a(units[slot])
        if 1 <= slot <= n:
            stage_bc(units[slot - 1])
        if slot >= 2:
            w = units[slot - 2]
            stage_def(w)
            if w[1] == 1:
                stage_g(w[0])


# ---------------------------------------------------------------------------
_CACHE = {}


def _get_nc(n_iter=1, variant=frozenset()):
    key = (n_iter, variant)
    if key not in _CACHE:
        _CACHE[key] = build(n_iter, variant)
    return _CACHE[key]


def make_in_maps(q_real, k_real, v_real, q_phase, k_phase, v_phase,
                 w_q, w_k, w_v, w_o):
    """Host-side shard + layout prep: per-core input dicts."""
    xt = {}
    for b in range(B):
        xt[("xqr", b)] = np.ascontiguousarray(q_real[b].T).astype(F16NP)
        xt[("xqp", b)] = np.ascontiguousarray(q_phase[b].T).astype(F16NP)
        xt[("xkr", b)] = np.ascontiguousarray(k_real[b].T).astype(F16NP)
        xt[("xkp", b)] = np.ascontiguousarray(k_phase[b].T).astype(F16NP)
        xt[("xvr", b)] = np.ascontiguousarray(v_real[b].T).astype(F16NP)
        xt[("xvp", b)] = np.ascontiguousarray(v_phase[b].T).astype(F16NP)
    wq16, wk16, wv16, wo16 = (w.astype(F16NP) for w in (w_q, w_k, w_v, w_o))
    in_maps = []
    for core in range(N_CORES):
        b, hg = divmod(core, HG)
        c0 = slice(hg * 2 * DK, hg * 2 * DK + DK)         # head h0 cols
        c1 = slice(hg * 2 * DK + DK, (hg + 1) * 2 * DK)   # head h1 cols
        def nsw(w):
            n = np.ascontiguousarray(np.concatenate([w[:, c0], w[:, c1]], 1))
            s = np.ascontiguousarray(np.concatenate([w[:, c1], w[:, c0]], 1))
            return n, s
        wqn, wqs = nsw(wq16)
        wkn, wks = nsw(wk16)
        wvn, wvs = nsw(wv16)
        wo_a = np.ascontiguousarray(np.concatenate([wo16[c0, :], wo16[c1, :]], 0))
        wo_b = np.ascontiguousarray(np.concatenate([wo16[c1, :], wo16[c0, :]], 0))
        in_maps.append({
            "xqr": xt[("xqr", b)], "xqp": xt[("xqp", b)],
            "xkr": xt[("xkr", b)], "xkp": xt[("xkp", b)],
            "xvr": xt[("xvr", b)], "xvp": xt[("xvp", b)],
            "wq_n": wqn, "wq_s": wqs,
            "wk_n": wkn, "wk_s": wks,
            "wv_n": wvn, "wv_s": wvs,
            "wo_a": wo_a, "wo_b": wo_b,
        })
    return in_maps


def gather_outputs(results):
    out_r = np.zeros((B, S, D), np.float32)
    out_p = np.zeros((B, S, D), np.float32)
    for core in range(N_CORES):
        b = core // HG
        out_r[b] += np.asarray(results[core]["o_r"], np.float32)
        out_p[b] += np.asarray(results[core]["o_p"], np.float32)
    return out_r, out_p


def _numpy_fallback(q_real, k_real, v_real, q_phase, k_phase, v_phase,
                    w_q, w_k, w_v, w_o, mask):
    def heads(x, w):
        y = x @ w
        return y.reshape(B, -1, H, DK).transpose(0, 2, 1, 3)
    qr, kr, vr = heads(q_real, w_q), heads(k_real, w_k), heads(v_real, w_v)
    qp, kp, vp = heads(q_phase, w_q), heads(k_phase, w_k), heads(v_phase, w_v)
    ar = np.einsum('bhqd,bhkd->bhqk', qr, kr) - np.einsum('bhqd,bhkd->bhqk', qp, kp)
    ap = np.einsum('bhqd,bhkd->bhqk', qr, kp) + np.einsum('bhqd,bhkd->bhqk', qp, kr)
    a = np.sqrt(ar * ar + ap * ap) / SCALE
    a = np.where(mask[:, None, :, :] == 0, np.float32(-1e9), a)
    a = a - a.max(axis=-1, keepdims=True)
    e = np.exp(a)
    a = e / e.sum(axis=-1, keepdims=True)
    xr = np.einsum('bhqk,bhkd->bhqd', a, vr).transpose(0, 2, 1, 3).reshape(B, -1, D)
    xp = np.einsum('bhqk,bhkd->bhqd', a, vp).transpose(0, 2, 1, 3).reshape(B, -1, D)
    return (xr @ w_o).astype(np.float32), (xp @ w_o).astype(np.float32)


def kernel(q_real, k_real, v_real, q_phase, k_phase, v_phase,
           w_q, w_k, w_v, w_o, mask):
    args = [np.asarray(a, np.float32) for a in
            (q_real, k_real, v_real, q_phase, k_phase, v_phase,
             w_q, w_k, w_v, w_o)]
    mask = np.asarray(mask)
    if not np.all(mask != 0):
        return _numpy_fallback(*args, mask)
    nc = _get_nc(1)
    in_maps = make_in_maps(*args)
    res = run_bass_kernel_spmd(nc, in_maps, core_ids=list(range(N_CORES)))
    return gather_outputs(res.results)


# revision 13
# speedup vs baseline: 1.1099x; 1.0247x over previous
"""Trainium2 Bass kernel for nn_MultiHeadAttention_65773129171319.

Complex-valued multi-head attention:
  attn = softmax(|Qc Kc^H| / sqrt(2 dk)) ; out = (attn @ Vr) Wo, (attn @ Vp) Wo

Sharding: 8 cores = 2 (batch) x 4 (head-groups of 2 heads).  Each core
computes its batch's full sequence for its 2 heads; the out-projection
partial sums (over head groups) are reduced on the host.

V2 design (all fp16 on device, fp32 PSUM):
  - Packed 2-head K/Q projections: one M=128 matmul per (plane, d-tile)
    computes both heads at once; plane pairs land in a single 2-bank PSUM
    tile, one ACT copy stages them to SBUF, and the GPSIMD (pool) engine
    repacks them into per-head stacked score operands:
      qc[0]=[Qr_h0;Qp_h0]  qc[1]=[Qp_h1;Qr_h1]
      kcr[0]=[Kr_h0;-Kp_h0] kcr[1]=[-Kp_h1;Kr_h1]
      kcp[0]=[Kp_h0;Kr_h0]  kcp[1]=[Kr_h1;Kp_h1]
    (The phase-plane matmuls use head-swapped weight copies so most of the
    repack copies are partition-offset-free.)
  - Scores per (strip, head): sT_r = kcr^T qc, sT_p = kcp^T qc as single
    C=128 matmuls producing transposed [sk, sq] tiles, two t-tiles per
    2-bank PSUM tile.  u = sT_r^2 + sT_p^2 via ACT Square + DVE SQADD.
  - m = sqrt(u) on ACT (the only table function -> zero table switches),
    attn = exp(m/SCALE) on DVE via custom EXP8 op:
      exp(m/s) ~= (((a*m + b)*m + c)^2)^2)^2   (max rel err ~5e-4)
  - Rowsums via ones-stationary matmul; reciprocal + partition broadcast;
    applied to the AV output (small side).
  - AV packed per head: stationary [vr_h|vp_h] (head1: [vp|vr]) gives
    M=128 AV matmuls; results scatter offset-free into xrT=[h0r;h1r] and
    xpT=[h1p;h0p]; out-projection uses wo_A (natural rows) for o_r and
    wo_B (head-swapped rows) for o_p.
  - Emission is software-pipelined over units w=(strip, head) in slots:
    scores(w) | sqrt+exp(w-1) | rowsum/AV/normalize(w-2), so PE, ACT and
    DVE always have a slot of ready work.
"""

import os
import sys

import numpy as np

try:
    import concourse.bass as bass
except ImportError:  # pragma: no cover
    sys.path.insert(0, "/opt/trn_rl_repo")
    import concourse.bass as bass

import concourse.mybir as mybir
import concourse.tile as tile
from concourse import bacc
from concourse.bass_utils import run_bass_kernel_spmd

B, S, D, H = 2, 2048, 512, 8
DK = D // H  # 64
SCALE = float((2 * DK) ** 0.5)  # sqrt(128)
P = 128
N_CORES = 8
HG = 4            # head groups (2 heads each)
DT = D // P       # 4 d-tiles for projection contraction
SKT = S // P      # 16 sk tiles
NSTRIP = 4        # sq strips of 512
STRIP = S // NSTRIP  # 512
NPAIR = SKT // 2  # t-pairs per (strip, head)

F32 = mybir.dt.float32
F16 = mybir.dt.float16
F16NP = np.float16

AF = mybir.ActivationFunctionType

# EXP8 poly coefficients: exp(m/SCALE) ~= (((EA*m+EB)*m+EC)^2^2)^2,
# fit on m in [0, 18] (empirical max |z| ~ 15.8); max rel err 5.3e-4.
EA = 6.734965764779986e-05
EB = 0.011003405951248851
EC = 1.0000654804195346


def register_custom_ops():
    """Register fused DVE ops (runtime extension of dve_ops.OPS)."""
    import concourse.dve_ops as dve_ops
    from concourse.dve_ops import DveOp
    from concourse.dve_spec import Spec, Src0, Src1, C0, C1, C2, sq, lower, _has_src1
    from concourse.dve_uop import DveOpSpec

    existing = {op.name: op for op in dve_ops.OPS}

    def mk(name, spec):
        if name in existing:
            return existing[name]
        row = max(dve_ops._SUB_OPCODE_FOR_NAME.values()) + 1
        assert row < 0x20, "no free DVE opcode rows"
        dve_ops._SUB_OPCODE_FOR_NAME[name] = row
        shas = {}
        for ver in ("v3", "v4"):
            s = DveOpSpec(name=name, opcode=row, uops=lower(spec, ver=ver),
                          rd1_en=_has_src1(spec))
            shas[ver] = s.sha(ver)
        op = DveOp(name, spec, subdim=False, uops_sha=shas)
        dve_ops.OPS.append(op)
        return op

    sqadd = mk("SQADD_ANT", Spec(
        body=sq(Src0) + Src1,
        reference=lambda in0, in1, s0, s1, imm2:
            in0.astype(np.float32) ** 2 + in1.astype(np.float32)))
    def _exp8_ref(in0, in1, s0, s1, imm2):
        p = (s0 * in0.astype(np.float32) + s1) * in0.astype(np.float32) + imm2
        return ((p ** 2) ** 2) ** 2

    exp8 = mk("EXP8_ANT", Spec(
        body=sq(sq(sq((Src0 * C0 + C1) * Src0 + C2))),
        reference=_exp8_ref))
    return sqadd, exp8


SQADD, EXP8 = register_custom_ops()


def build(n_iter: int = 1, variant: frozenset = frozenset()):
    """Build (and bacc-compile) the per-core SPMD program."""
    nc = bacc.Bacc("TRN2", target_bir_lowering=False, debug=False,
                   num_devices=N_CORES)

    dr = {}
    for name in ("xqr", "xqp", "xkr", "xkp", "xvr", "xvp"):
        dr[name] = nc.dram_tensor(name, [D, S], F16, kind="ExternalInput")
    for name in ("wq_n", "wq_s", "wk_n", "wk_s", "wv_n", "wv_s"):
        dr[name] = nc.dram_tensor(name, [D, 2 * DK], F16, kind="ExternalInput")
    dr["wo_a"] = nc.dram_tensor("wo_a", [2 * DK, D], F16, kind="ExternalInput")
    dr["wo_b"] = nc.dram_tensor("wo_b", [2 * DK, D], F16, kind="ExternalInput")
    dr["o_r"] = nc.dram_tensor("o_r", [S, D], F16, kind="ExternalOutput")
    dr["o_p"] = nc.dram_tensor("o_p", [S, D], F16, kind="ExternalOutput")

    with tile.TileContext(nc) as tc:
        _emit(tc, dr, n_iter, variant)
    nc.compile()
    return nc


def _emit(tc, dr, n_iter, variant=frozenset()):
    from contextlib import ExitStack

    ctx = ExitStack()
    with ctx:
        pools = dict(
            singles=ctx.enter_context(tc.tile_pool(name="singles", bufs=1)),
            xpool=ctx.enter_context(tc.tile_pool(name="xp", bufs=3)),
            ppool=ctx.enter_context(tc.tile_pool(name="pp", bufs=2)),
            upool=ctx.enter_context(tc.tile_pool(name="up", bufs=4)),
            rbpool=ctx.enter_context(tc.tile_pool(name="rb", bufs=2)),
            rrpool=ctx.enter_context(tc.tile_pool(name="rr", bufs=2)),
            opool=ctx.enter_context(tc.tile_pool(name="op", bufs=3)),
            psA=ctx.enter_context(tc.tile_pool(name="psA", bufs=4, space="PSUM")),
            psRS=ctx.enter_context(tc.tile_pool(name="psRS", bufs=1, space="PSUM")),
            psAV=ctx.enter_context(tc.tile_pool(name="psAV", bufs=2, space="PSUM")),
            psO=ctx.enter_context(tc.tile_pool(name="psO", bufs=1, space="PSUM")),
        )
        if n_iter > 1:
            with tc.For_i(0, n_iter, 1):
                _body(tc, dr, variant, **pools)
        else:
            _body(tc, dr, variant, **pools)


def _body(tc, dr, variant, singles, xpool, ppool, upool, rbpool, rrpool,
          opool, psA, psRS, psAV, psO):
    nc = tc.nc
    V = lambda name: name in variant

    # ---- weights to SBUF -------------------------------------------------
    wsb = {}
    for name in ("wq_n", "wq_s", "wk_n", "wk_s", "wv_n", "wv_s"):
        t = singles.tile([P, DT, 2 * DK], F16, tag=f"w_{name}", name=f"w_{name}")
        nc.sync.dma_start(out=t[:], in_=dr[name].rearrange("(dt p) m -> p dt m", p=P))
        wsb[name] = t
    wo_a = singles.tile([P, D], F16, tag="w_wo_a", name="w_wo_a")
    nc.sync.dma_start(out=wo_a[:], in_=dr["wo_a"][:])
    wo_b = singles.tile([P, D], F16, tag="w_wo_b", name="w_wo_b")
    nc.sync.dma_start(out=wo_b[:], in_=dr["wo_b"][:])
    ones = singles.tile([P, 1], F16, tag="ones", name="ones")
    nc.vector.memset(ones[:], 1.0)

    # ---- persistent SBUF tensors ----------------------------------------
    kcr = [singles.tile([P, S], F16, tag=f"kcr{h}", name=f"kcr{h}") for h in range(2)]
    kcp = [singles.tile([P, S], F16, tag=f"kcp{h}", name=f"kcp{h}") for h in range(2)]
    qc = [singles.tile([P, S], F16, tag=f"qc{h}", name=f"qc{h}") for h in range(2)]
    v2 = [singles.tile([P, SKT, P], F16, tag=f"v2_{h}", name=f"v2_{h}") for h in range(2)]
    xrT = singles.tile([P, S], F16, tag="xrT", name="xrT")
    xpT = singles.tile([P, S], F16, tag="xpT", name="xpT")

    def _xdma(out, in_):
        if not V("nodma"):
            nc.sync.dma_start(out=out, in_=in_)

    def _xs(name, ssl):
        t = xpool.tile([P, DT, STRIP], F16, tag="xs", name="xs")
        _xdma(t[:], dr[name].rearrange("(dt p) s -> p dt s", p=P)[:, :, ssl])
        return t

    # ---- K projection + repack ------------------------------------------
    def k_proj(s):
        ssl = slice(s * STRIP, (s + 1) * STRIP)
        xtr = _xs("xkr", ssl)
        xtp = _xs("xkp", ssl)
        psKa = psA.tile([P, STRIP], F32, tag="psA", name="psA")
        psKb = psA.tile([P, STRIP], F32, tag="psA", name="psA")
        if not V("noproj"):
            for dt in range(DT):
                st, sp = (dt == 0), (dt == DT - 1)
                nc.tensor.matmul(psKa[:], wsb["wk_n"][:, dt, :],
                                 xtr[:, dt, :], start=st, stop=sp)
                nc.tensor.matmul(psKb[:], wsb["wk_s"][:, dt, :],
                                 xtp[:, dt, :], start=st, stop=sp)
        pl = ppool.tile([P, 2, STRIP], F16, tag="pl", name="pl")
        nc.scalar.copy(pl[:, 0, :], psKa[:])
        nc.scalar.copy(pl[:, 1, :], psKb[:])
        lo, hi = slice(0, DK), slice(DK, P)
        gp = nc.gpsimd
        # plane 0 = [Kr_h0; Kr_h1], plane 1 = [Kp_h1; Kp_h0] (partition halves)
        gp.tensor_copy(kcr[0][lo, ssl], pl[lo, 0, :])
        gp.tensor_scalar_mul(kcr[0][hi, ssl], pl[hi, 1, :], -1.0)
        gp.tensor_scalar_mul(kcr[1][lo, ssl], pl[lo, 1, :], -1.0)
        gp.tensor_copy(kcr[1][hi, ssl], pl[hi, 0, :])
        gp.tensor_copy(kcp[0][lo, ssl], pl[hi, 1, :])
        gp.tensor_copy(kcp[0][hi, ssl], pl[lo, 0, :])
        gp.tensor_copy(kcp[1][lo, ssl], pl[hi, 0, :])
        gp.tensor_copy(kcp[1][hi, ssl], pl[lo, 1, :])

    # ---- Q projection + repack ------------------------------------------
    def q_proj(s):
        ssl = slice(s * STRIP, (s + 1) * STRIP)
        xtr = _xs("xqr", ssl)
        xtp = _xs("xqp", ssl)
        psQa = psA.tile([P, STRIP], F32, tag="psA", name="psA")
        psQb = psA.tile([P, STRIP], F32, tag="psA", name="psA")
        if not V("noproj"):
            for dt in range(DT):
                st, sp = (dt == 0), (dt == DT - 1)
                nc.tensor.matmul(psQa[:], wsb["wq_n"][:, dt, :],
                                 xtr[:, dt, :], start=st, stop=sp)
                nc.tensor.matmul(psQb[:], wsb["wq_s"][:, dt, :],
                                 xtp[:, dt, :], start=st, stop=sp)
        pl = ppool.tile([P, 2, STRIP], F16, tag="pl", name="pl")
        nc.scalar.copy(pl[:, 0, :], psQa[:])
        nc.scalar.copy(pl[:, 1, :], psQb[:])
        lo, hi = slice(0, DK), slice(DK, P)
        gp = nc.gpsimd
        # plane 0 = [Qr_h0; Qr_h1], plane 1 = [Qp_h1; Qp_h0]
        gp.tensor_copy(qc[0][lo, ssl], pl[lo, 0, :])
        gp.tensor_copy(qc[0][hi, ssl], pl[hi, 1, :])
        gp.tensor_copy(qc[1][lo, ssl], pl[lo, 1, :])
        gp.tensor_copy(qc[1][hi, ssl], pl[hi, 0, :])

    # ---- V projection ----------------------------------------------------
    def v_proj(s):
        xvr_t = _xs("xvr", slice(s * STRIP, (s + 1) * STRIP))
        xvp_t = _xs("xvp", slice(s * STRIP, (s + 1) * STRIP))
        for tt in range(STRIP // P):
            t = s * (STRIP // P) + tt
            tpsl = slice(tt * P, (tt + 1) * P)
            psVa = psA.tile([P, STRIP], F32, tag="psA", name="psA")
            psVb = psA.tile([P, STRIP], F32, tag="psA", name="psA")
            if not V("noproj"):
                for dt in range(DT):
                    st, sp = (dt == 0), (dt == DT - 1)
                    nc.tensor.matmul(psVa[:, 0:2 * DK], xvr_t[:, dt, tpsl],
                                     wsb["wv_n"][:, dt, :], start=st, stop=sp)
                    nc.tensor.matmul(psVb[:, 0:2 * DK], xvp_t[:, dt, tpsl],
                                     wsb["wv_s"][:, dt, :], start=st, stop=sp)
            # psVa = [vr_h0 | vr_h1], psVb = [vp_h1 | vp_h0]
            nc.scalar.copy(v2[0][:, t, 0:DK], psVa[:, 0:DK])
            nc.scalar.copy(v2[0][:, t, DK:2 * DK], psVb[:, DK:2 * DK])
            nc.scalar.copy(v2[1][:, t, 0:DK], psVb[:, 0:DK])
            nc.scalar.copy(v2[1][:, t, DK:2 * DK], psVa[:, DK:2 * DK])

    # ---- attention stages -----------------------------------------------
    units = [(s, h) for s in range(NSTRIP) for h in range(2)]
    u_tiles = {}

    def stage_a(w):
        s, h = w
        ssl = slice(s * STRIP, (s + 1) * STRIP)
        u = upool.tile([P, SKT, STRIP], F16, tag="u", name="u")
        u_tiles[w] = u
        for t in range(SKT):
            tsl = slice(t * P, (t + 1) * P)
            psr = psA.tile([P, STRIP], F32, tag="psA", name="psA")
            psp = psA.tile([P, STRIP], F32, tag="psA", name="psA")
            if not V("noscores"):
                nc.tensor.matmul(psr[:], kcr[h][:, tsl], qc[h][:, ssl],
                                 start=True, stop=True)
                nc.tensor.matmul(psp[:], kcp[h][:, tsl], qc[h][:, ssl],
                                 start=True, stop=True)
            usl = u[:, t, :]
            if V("nosq"):
                if t == 0:
                    nc.vector.memset(u[:], 1.0)
                continue
            nc.scalar.square(usl, psr[:])
            nc.vector._custom_dve(SQADD, out=usl, in0=psp[:], in1=usl)

    def stage_bc(w):
        if V("nosqrtexp"):
            return
        u = u_tiles[w]
        half = SKT // 2
        for c in range(2):
            csl = u[:, c * half:(c + 1) * half, :]
            nc.scalar.activation(csl, csl, AF.Sqrt)
        for c in range(2):
            csl = u[:, c * half:(c + 1) * half, :]
            nc.vector._custom_dve(EXP8, out=csl, in0=csl, s0=EA, s1=EB, imm2=EC)

    def stage_def(w):
        s, h = w
        ssl = slice(s * STRIP, (s + 1) * STRIP)
        attn = u_tiles.pop(w)
        ps_rs = psRS.tile([1, STRIP], F32, tag="rs", name="rs")
        if not V("norowsum"):
            for t in range(SKT):
                nc.tensor.matmul(ps_rs[0:1, :], ones[:], attn[:, t, :],
                                 start=(t == 0), stop=(t == SKT - 1))
        rrec = rrpool.tile([1, STRIP], F32, tag="rrec", name="rrec")
        if V("norecip"):
            nc.vector.memset(rrec[:], 1.0)
        else:
            nc.vector.reciprocal_approx_fast(rrec[:], ps_rs[0:1, :])
        rb = rbpool.tile([P, STRIP], F32, tag="rb", name="rb")
        if V("nobcast"):
            nc.vector.memset(rb[:], 1.0)
        else:
            nc.gpsimd.partition_broadcast(rb[:], rrec[:])
        ps_av = psAV.tile([P, STRIP], F32, tag="av", name="av")
        if not V("noav"):
            for t in range(SKT):
                nc.tensor.matmul(ps_av[:], v2[h][:, t, :], attn[:, t, :],
                                 start=(t == 0), stop=(t == SKT - 1))
        lo, hi = slice(0, DK), slice(DK, P)
        if h == 0:
            nc.vector.tensor_mul(xrT[lo, ssl], ps_av[lo, :], rb[lo, :])
            nc.vector.tensor_mul(xpT[hi, ssl], ps_av[hi, :], rb[hi, :])
        else:
            nc.vector.tensor_mul(xpT[lo, ssl], ps_av[lo, :], rb[lo, :])
            nc.vector.tensor_mul(xrT[hi, ssl], ps_av[hi, :], rb[hi, :])

    def stage_g(s):
        if V("noout"):
            return
        for q in range(STRIP // P):
            qsl = slice((s * (STRIP // P) + q) * P, (s * (STRIP // P) + q + 1) * P)
            for xT, wo, out in ((xrT, wo_a, dr["o_r"]), (xpT, wo_b, dr["o_p"])):
                ps_o = psO.tile([P, D], F32, tag="o", name="o")
                nc.tensor.matmul(ps_o[:], xT[:, qsl], wo[:], start=True, stop=True)
                osb = opool.tile([P, D], F16, tag="osb", name="osb")
                nc.scalar.copy(osb[:], ps_o[:])
                _xdma(out[qsl, :], osb[:])

    # ---- emission: projections then software-pipelined attention --------
    for s in range(NSTRIP):
        k_proj(s)
    for s in range(NSTRIP):
        q_proj(s)
    for s in range(NSTRIP):
        v_proj(s)

    n = len(units)
    for slot in range(n + 2):
        if slot < n:
            stage_a(units[slot])
        if 1 <= slot <= n:
            stage_bc(units[slot - 1])
        if slot >= 2:
            w = units[slot - 2]
            stage_def(w)
            if w[1] == 1:
                stage_g(w[0])


# ---------------------------------------------------------------------------
_CACHE = {}


def _get_nc(n_iter=1, variant=frozenset()):
    key = (n_iter, variant)
    if key not in _CACHE:
        _CACHE[key] = build(n_iter, variant)
    return _CACHE[key]


def make_in_maps(q_real, k_real, v_real, q_phase, k_phase, v_phase,
                 w_q, w_k, w_v, w_o):
    """Host-side shard + layout prep: per-core input dicts."""
    xt = {}
    for b in range(B):
        xt[("xqr", b)] = np.ascontiguousarray(q_real[b].T).astype(F16NP)
        xt[("xqp", b)] = np.ascontiguousarray(q_phase[b].T).astype(F16NP)
        xt[("xkr", b)] = np.ascontiguousarray(k_real[b].T).astype(F16NP)
        xt[("xkp", b)] = np.ascontiguousarray(k_phase[b].T).astype(F16NP)
        xt[("xvr", b)] = np.ascontiguousarray(v_real[b].T).astype(F16NP)
        xt[("xvp", b)] = np.ascontiguousarray(v_phase[b].T).astype(F16NP)
    wq16, wk16, wv16, wo16 = (w.astype(F16NP) for w in (w_q, w_k, w_v, w_o))
    in_maps = []
    for core in range(N_CORES):
        b, hg = divmod(core, HG)
        c0 = slice(hg * 2 * DK, hg * 2 * DK + DK)         # head h0 cols
        c1 = slice(hg * 2 * DK + DK, (hg + 1) * 2 * DK)   # head h1 cols
        def nsw(w):
            n = np.ascontiguousarray(np.concatenate([w[:, c0], w[:, c1]], 1))
            s = np.ascontiguousarray(np.concatenate([w[:, c1], w[:, c0]], 1))
            return n, s
        wqn, wqs = nsw(wq16)
        wkn, wks = nsw(wk16)
        wvn, wvs = nsw(wv16)
        wo_a = np.ascontiguousarray(np.concatenate([wo16[c0, :], wo16[c1, :]], 0))
        wo_b = np.ascontiguousarray(np.concatenate([wo16[c1, :], wo16[c0, :]], 0))
        in_maps.append({
            "xqr": xt[("xqr", b)], "xqp": xt[("xqp", b)],
            "xkr": xt[("xkr", b)], "xkp": xt[("xkp", b)],
            "xvr": xt[("xvr", b)], "xvp": xt[("xvp", b)],
            "wq_n": wqn, "wq_s": wqs,
            "wk_n": wkn, "wk_s": wks,
            "wv_n": wvn, "wv_s": wvs,
            "wo_a": wo_a, "wo_b": wo_b,
        })
    return in_maps


def gather_outputs(results):
    out_r = np.zeros((B, S, D), np.float32)
    out_p = np.zeros((B, S, D), np.float32)
    for core in range(N_CORES):
        b = core // HG
        out_r[b] += np.asarray(results[core]["o_r"], np.float32)
        out_p[b] += np.asarray(results[core]["o_p"], np.float32)
    return out_r, out_p


def _numpy_fallback(q_real, k_real, v_real, q_phase, k_phase, v_phase,
                    w_q, w_k, w_v, w_o, mask):
    def heads(x, w):
        y = x @ w
        return y.reshape(B, -1, H, DK).transpose(0, 2, 1, 3)
    qr, kr, vr = heads(q_real, w_q), heads(k_real, w_k), heads(v_real, w_v)
    qp, kp, vp = heads(q_phase, w_q), heads(k_phase, w_k), heads(v_phase, w_v)
    ar = np.einsum('bhqd,bhkd->bhqk', qr, kr) - np.einsum('bhqd,bhkd->bhqk', qp, kp)
    ap = np.einsum('bhqd,bhkd->bhqk', qr, kp) + np.einsum('bhqd,bhkd->bhqk', qp, kr)
    a = np.sqrt(ar * ar + ap * ap) / SCALE
    a = np.where(mask[:, None, :, :] == 0, np.float32(-1e9), a)
    a = a - a.max(axis=-1, keepdims=True)
    e = np.exp(a)
    a = e / e.sum(axis=-1, keepdims=True)
    xr = np.einsum('bhqk,bhkd->bhqd', a, vr).transpose(0, 2, 1, 3).reshape(B, -1, D)
    xp = np.einsum('bhqk,bhkd->bhqd', a, vp).transpose(0, 2, 1, 3).reshape(B, -1, D)
    return (xr @ w_o).astype(np.float32), (xp @ w_o).astype(np.float32)


def kernel(q_real, k_real, v_real, q_phase, k_phase, v_phase,
           w_q, w_k, w_v, w_o, mask):
    args = [np.asarray(a, np.float32) for a in
            (q_real, k_real, v_real, q_phase, k_phase, v_phase,
             w_q, w_k, w_v, w_o)]
    mask = np.asarray(mask)
    if not np.all(mask != 0):
        return _numpy_fallback(*args, mask)
    nc = _get_nc(1)
    in_maps = make_in_maps(*args)
    res = run_bass_kernel_spmd(nc, in_maps, core_ids=list(range(N_CORES)))
    return gather_outputs(res.results)


# revision 28
# speedup vs baseline: 1.3199x; 1.1892x over previous
"""Trainium2 Bass kernel for nn_MultiHeadAttention_65773129171319.

Complex-valued multi-head attention:
  attn = softmax(|Qc Kc^H| / sqrt(2 dk)) ; out = (attn @ Vr) Wo, (attn @ Vp) Wo

Sharding: 8 cores = 2 (batch) x 4 (head-groups of 2 heads).  Each core
computes its batch's full sequence for its 2 heads; the out-projection
partial sums (over head groups) are reduced on the host.

V2 design (all fp16 on device, fp32 PSUM):
  - Packed 2-head K/Q projections: one M=128 matmul per (plane, d-tile)
    computes both heads at once; plane pairs land in a single 2-bank PSUM
    tile, one ACT copy stages them to SBUF, and the GPSIMD (pool) engine
    repacks them into per-head stacked score operands:
      qc[0]=[Qr_h0;Qp_h0]  qc[1]=[Qp_h1;Qr_h1]
      kcr[0]=[Kr_h0;-Kp_h0] kcr[1]=[-Kp_h1;Kr_h1]
      kcp[0]=[Kp_h0;Kr_h0]  kcp[1]=[Kr_h1;Kp_h1]
    (The phase-plane matmuls use head-swapped weight copies so most of the
    repack copies are partition-offset-free.)
  - Scores per (strip, head): sT_r = kcr^T qc, sT_p = kcp^T qc as single
    C=128 matmuls producing transposed [sk, sq] tiles, two t-tiles per
    2-bank PSUM tile.  u = sT_r^2 + sT_p^2 via ACT Square + DVE SQADD.
  - m = sqrt(u) on ACT (the only table function -> zero table switches),
    attn = exp(m/SCALE) on DVE via custom EXP8 op:
      exp(m/s) ~= (((a*m + b)*m + c)^2)^2)^2   (max rel err ~5e-4)
  - Rowsums via ones-stationary matmul; reciprocal + partition broadcast;
    applied to the AV output (small side).
  - AV packed per head: stationary [vr_h|vp_h] (head1: [vp|vr]) gives
    M=128 AV matmuls; results scatter offset-free into xrT=[h0r;h1r] and
    xpT=[h1p;h0p]; out-projection uses wo_A (natural rows) for o_r and
    wo_B (head-swapped rows) for o_p.
  - Emission is software-pipelined over units w=(strip, head) in slots:
    scores(w) | sqrt+exp(w-1) | rowsum/AV/normalize(w-2), so PE, ACT and
    DVE always have a slot of ready work.
"""

import os
import sys

import numpy as np

try:
    import concourse.bass as bass
except ImportError:  # pragma: no cover
    sys.path.insert(0, "/opt/trn_rl_repo")
    import concourse.bass as bass

import concourse.mybir as mybir
import concourse.tile as tile
from concourse import bacc
from concourse.bass_utils import run_bass_kernel_spmd

B, S, D, H = 2, 2048, 512, 8
DK = D // H  # 64
SCALE = float((2 * DK) ** 0.5)  # sqrt(128)
P = 128
N_CORES = 8
HG = 4            # head groups (2 heads each)
DT = D // P       # 4 d-tiles for projection contraction
SKT = S // P      # 16 sk tiles
NSTRIP = 4        # sq strips of 512
STRIP = S // NSTRIP  # 512
NPAIR = SKT // 2  # t-pairs per (strip, head)

F32 = mybir.dt.float32
F16 = mybir.dt.float16
F16NP = np.float16

AF = mybir.ActivationFunctionType

# EXP8 poly coefficients: exp(m/SCALE) ~= (((EA*m+EB)*m+EC)^2^2)^2,
# fit on m in [0, 18] (empirical max |z| ~ 15.8); max rel err 5.3e-4.
EA = 6.734965764779986e-05
EB = 0.011003405951248851
EC = 1.0000654804195346


def register_custom_ops():
    """Register fused DVE ops (runtime extension of dve_ops.OPS)."""
    import concourse.dve_ops as dve_ops
    from concourse.dve_ops import DveOp
    from concourse.dve_spec import Spec, Src0, Src1, C0, C1, C2, sq, lower, _has_src1
    from concourse.dve_uop import DveOpSpec

    existing = {op.name: op for op in dve_ops.OPS}

    def mk(name, spec):
        if name in existing:
            return existing[name]
        row = max(dve_ops._SUB_OPCODE_FOR_NAME.values()) + 1
        assert row < 0x20, "no free DVE opcode rows"
        dve_ops._SUB_OPCODE_FOR_NAME[name] = row
        shas = {}
        for ver in ("v3", "v4"):
            s = DveOpSpec(name=name, opcode=row, uops=lower(spec, ver=ver),
                          rd1_en=_has_src1(spec))
            shas[ver] = s.sha(ver)
        op = DveOp(name, spec, subdim=False, uops_sha=shas)
        dve_ops.OPS.append(op)
        return op

    sqadd = mk("SQADD_ANT", Spec(
        body=sq(Src0) + Src1,
        reference=lambda in0, in1, s0, s1, imm2:
            in0.astype(np.float32) ** 2 + in1.astype(np.float32)))
    def _exp8_ref(in0, in1, s0, s1, imm2):
        p = (s0 * in0.astype(np.float32) + s1) * in0.astype(np.float32) + imm2
        return ((p ** 2) ** 2) ** 2

    exp8 = mk("EXP8_ANT", Spec(
        body=sq(sq(sq((Src0 * C0 + C1) * Src0 + C2))),
        reference=_exp8_ref))
    return sqadd, exp8


SQADD, EXP8 = register_custom_ops()


def build(n_iter: int = 1, variant: frozenset = frozenset()):
    """Build (and bacc-compile) the per-core SPMD program."""
    nc = bacc.Bacc("TRN2", target_bir_lowering=False, debug=False,
                   num_devices=N_CORES)

    dr = {}
    for name in ("xqr", "xqp", "xkr", "xkp", "xvr", "xvp"):
        dr[name] = nc.dram_tensor(name, [D, S], F16, kind="ExternalInput")
    for name in ("wq_n", "wq_s", "wk_n", "wk_s", "wv_n", "wv_s"):
        dr[name] = nc.dram_tensor(name, [D, 2 * DK], F16, kind="ExternalInput")
    dr["wo_a"] = nc.dram_tensor("wo_a", [2 * DK, D], F16, kind="ExternalInput")
    dr["wo_b"] = nc.dram_tensor("wo_b", [2 * DK, D], F16, kind="ExternalInput")
    dr["o_r"] = nc.dram_tensor("o_r", [S, D], F16, kind="ExternalOutput")
    dr["o_p"] = nc.dram_tensor("o_p", [S, D], F16, kind="ExternalOutput")

    with tile.TileContext(nc) as tc:
        _emit(tc, dr, n_iter, variant)
    nc.compile()
    return nc


def _emit(tc, dr, n_iter, variant=frozenset()):
    from contextlib import ExitStack

    ctx = ExitStack()
    with ctx:
        pools = dict(
            singles=ctx.enter_context(tc.tile_pool(name="singles", bufs=2)),
            xpool=ctx.enter_context(tc.tile_pool(name="xp", bufs=3)),
            ppool=ctx.enter_context(tc.tile_pool(name="pp", bufs=2)),
            upool=ctx.enter_context(tc.tile_pool(name="up", bufs=4)),
            rbpool=ctx.enter_context(tc.tile_pool(name="rb", bufs=2)),
            rrpool=ctx.enter_context(tc.tile_pool(name="rr", bufs=2)),
            opool=ctx.enter_context(tc.tile_pool(name="op", bufs=3)),
            psA=ctx.enter_context(tc.tile_pool(name="psA", bufs=4, space="PSUM")),
            psRS=ctx.enter_context(tc.tile_pool(name="psRS", bufs=1, space="PSUM")),
            psAV=ctx.enter_context(tc.tile_pool(name="psAV", bufs=2, space="PSUM")),
            psO=ctx.enter_context(tc.tile_pool(name="psO", bufs=1, space="PSUM")),
        )
        if n_iter > 1:
            # unroll x2 inside the hw loop so consecutive iterations use
            # alternating persistent buffers (singles bufs=2) and overlap.
            assert n_iter % 2 == 0, "n_iter must be even (or 1)"
            with tc.For_i(0, n_iter // 2, 1):
                _body(tc, dr, variant, **pools)
                _body(tc, dr, variant, **pools)
        else:
            _body(tc, dr, variant, **pools)


def _body(tc, dr, variant, singles, xpool, ppool, upool, rbpool, rrpool,
          opool, psA, psRS, psAV, psO):
    nc = tc.nc
    V = lambda name: name in variant

    # ---- weights to SBUF -------------------------------------------------
    wsb = {}
    for name in ("wq_n", "wq_s", "wk_n", "wk_s", "wv_n", "wv_s"):
        t = singles.tile([P, DT, 2 * DK], F16, tag=f"w_{name}", name=f"w_{name}")
        nc.sync.dma_start(out=t[:], in_=dr[name].rearrange("(dt p) m -> p dt m", p=P))
        wsb[name] = t
    wo_a = singles.tile([P, D], F16, tag="w_wo_a", name="w_wo_a")
    nc.sync.dma_start(out=wo_a[:], in_=dr["wo_a"][:])
    wo_b = singles.tile([P, D], F16, tag="w_wo_b", name="w_wo_b")
    nc.sync.dma_start(out=wo_b[:], in_=dr["wo_b"][:])
    ones = singles.tile([P, 1], F16, tag="ones", name="ones")
    nc.vector.memset(ones[:], 1.0)

    # ---- persistent SBUF tensors ----------------------------------------
    kcr = [singles.tile([P, S], F16, tag=f"kcr{h}", name=f"kcr{h}") for h in range(2)]
    kcp = [singles.tile([P, S], F16, tag=f"kcp{h}", name=f"kcp{h}") for h in range(2)]
    qc = [singles.tile([P, S], F16, tag=f"qc{h}", name=f"qc{h}") for h in range(2)]
    v2 = [singles.tile([P, SKT, P], F16, tag=f"v2_{h}", name=f"v2_{h}") for h in range(2)]
    xrT = singles.tile([P, S], F16, tag="xrT", name="xrT")
    xpT = singles.tile([P, S], F16, tag="xpT", name="xpT")

    def _xdma(out, in_):
        if not V("nodma"):
            nc.sync.dma_start(out=out, in_=in_)

    def _xs(name, ssl):
        t = xpool.tile([P, DT, STRIP], F16, tag="xs", name="xs")
        _xdma(t[:], dr[name].rearrange("(dt p) s -> p dt s", p=P)[:, :, ssl])
        return t

    # ---- K projection + repack ------------------------------------------
    def k_proj(s):
        ssl = slice(s * STRIP, (s + 1) * STRIP)
        xtr = _xs("xkr", ssl)
        xtp = _xs("xkp", ssl)
        psKa = psA.tile([P, STRIP], F32, tag="psA", name="psA")
        psKb = psA.tile([P, STRIP], F32, tag="psA", name="psA")
        if not V("noproj"):
            for dt in range(DT):
                st, sp = (dt == 0), (dt == DT - 1)
                nc.tensor.matmul(psKa[:], wsb["wk_n"][:, dt, :],
                                 xtr[:, dt, :], start=st, stop=sp)
                nc.tensor.matmul(psKb[:], wsb["wk_s"][:, dt, :],
                                 xtp[:, dt, :], start=st, stop=sp)
        pl = ppool.tile([P, 2, STRIP], F16, tag="pl", name="pl")
        nc.scalar.copy(pl[:, 0, :], psKa[:])
        nc.scalar.copy(pl[:, 1, :], psKb[:])
        lo, hi = slice(0, DK), slice(DK, P)
        dv = nc.gpsimd
        # plane 0 = [Kr_h0; Kr_h1], plane 1 = [Kp_h1; Kp_h0] (partition halves)
        dv.tensor_copy(kcr[0][lo, ssl], pl[lo, 0, :])
        dv.tensor_scalar_mul(kcr[0][hi, ssl], pl[hi, 1, :], -1.0)
        dv.tensor_scalar_mul(kcr[1][lo, ssl], pl[lo, 1, :], -1.0)
        dv.tensor_copy(kcr[1][hi, ssl], pl[hi, 0, :])
        dv.tensor_copy(kcp[0][lo, ssl], pl[hi, 1, :])
        dv.tensor_copy(kcp[0][hi, ssl], pl[lo, 0, :])
        dv.tensor_copy(kcp[1][lo, ssl], pl[hi, 0, :])
        dv.tensor_copy(kcp[1][hi, ssl], pl[lo, 1, :])

    # ---- Q projection + repack ------------------------------------------
    def q_proj(s):
        ssl = slice(s * STRIP, (s + 1) * STRIP)
        xtr = _xs("xqr", ssl)
        xtp = _xs("xqp", ssl)
        psQa = psA.tile([P, STRIP], F32, tag="psA", name="psA")
        psQb = psA.tile([P, STRIP], F32, tag="psA", name="psA")
        if not V("noproj"):
            for dt in range(DT):
                st, sp = (dt == 0), (dt == DT - 1)
                nc.tensor.matmul(psQa[:], wsb["wq_n"][:, dt, :],
                                 xtr[:, dt, :], start=st, stop=sp)
                nc.tensor.matmul(psQb[:], wsb["wq_s"][:, dt, :],
                                 xtp[:, dt, :], start=st, stop=sp)
        pl = ppool.tile([P, 2, STRIP], F16, tag="pl", name="pl")
        nc.scalar.copy(pl[:, 0, :], psQa[:])
        nc.scalar.copy(pl[:, 1, :], psQb[:])
        lo, hi = slice(0, DK), slice(DK, P)
        dv = nc.gpsimd
        # plane 0 = [Qr_h0; Qr_h1], plane 1 = [Qp_h1; Qp_h0]
        dv.tensor_copy(qc[0][lo, ssl], pl[lo, 0, :])
        dv.tensor_copy(qc[0][hi, ssl], pl[hi, 1, :])
        dv.tensor_copy(qc[1][lo, ssl], pl[lo, 1, :])
        dv.tensor_copy(qc[1][hi, ssl], pl[hi, 0, :])

    # ---- V projection ----------------------------------------------------
    def v_proj(s):
        xvr_t = _xs("xvr", slice(s * STRIP, (s + 1) * STRIP))
        xvp_t = _xs("xvp", slice(s * STRIP, (s + 1) * STRIP))
        for tt in range(STRIP // P):
            t = s * (STRIP // P) + tt
            tpsl = slice(tt * P, (tt + 1) * P)
            psVa = psA.tile([P, STRIP], F32, tag="psA", name="psA")
            psVb = psA.tile([P, STRIP], F32, tag="psA", name="psA")
            if not V("noproj"):
                for dt in range(DT):
                    st, sp = (dt == 0), (dt == DT - 1)
                    nc.tensor.matmul(psVa[:, 0:2 * DK], xvr_t[:, dt, tpsl],
                                     wsb["wv_n"][:, dt, :], start=st, stop=sp)
                    nc.tensor.matmul(psVb[:, 0:2 * DK], xvp_t[:, dt, tpsl],
                                     wsb["wv_s"][:, dt, :], start=st, stop=sp)
            # psVa = [vr_h0 | vr_h1], psVb = [vp_h1 | vp_h0]
            nc.scalar.copy(v2[0][:, t, 0:DK], psVa[:, 0:DK])
            nc.vector.tensor_copy(v2[0][:, t, DK:2 * DK], psVb[:, DK:2 * DK])
            nc.scalar.copy(v2[1][:, t, 0:DK], psVb[:, 0:DK])
            nc.vector.tensor_copy(v2[1][:, t, DK:2 * DK], psVa[:, DK:2 * DK])

    # ---- attention stages -----------------------------------------------
    units = [(s, h) for s in range(NSTRIP) for h in range(2)]
    u_tiles = {}

    def stage_g(s):
        if V("noout"):
            return
        for q in range(STRIP // P):
            qsl = slice((s * (STRIP // P) + q) * P, (s * (STRIP // P) + q + 1) * P)
            for xT, wo, out in ((xrT, wo_a, dr["o_r"]), (xpT, wo_b, dr["o_p"])):
                ps_o = psO.tile([P, D], F32, tag="o", name="o")
                nc.tensor.matmul(ps_o[:], xT[:, qsl], wo[:], start=True, stop=True)
                osb = opool.tile([P, D], F16, tag="osb", name="osb")
                nc.scalar.copy(osb[:], ps_o[:])
                # output DMA on the pool queue so the SP queue stays free to
                # prefetch the next iteration's inputs.
                if not V("nodma"):
                    nc.gpsimd.dma_start(out=out[qsl, :], in_=osb[:])

    def slot_emit(wA, wBC, wDEF):
        """One pipeline slot, interleaved per sk-tile t so no engine sees a
        long burst of another stage:
          wA:   scores + square + sqadd        (PE + ACT + DVE)
          wBC:  sqrt + exp chunks of wA's predecessor (ACT + DVE)
          wDEF: rowsum + AV + normalize of the unit before that (PE + DVE)
        """
        if wA is not None:
            sA, hA = wA
            sslA = slice(sA * STRIP, (sA + 1) * STRIP)
            uA = upool.tile([P, SKT, STRIP], F16, tag="u", name="u")
            u_tiles[wA] = uA
            if V("nosq"):
                nc.vector.memset(uA[:], 1.0)
        if wDEF is not None:
            sD, hD = wDEF
            sslD = slice(sD * STRIP, (sD + 1) * STRIP)
            attn = u_tiles.pop(wDEF)
            ps_rs = psRS.tile([1, STRIP], F32, tag="rs", name="rs")
            ps_av = psAV.tile([P, STRIP], F32, tag="av", name="av")
        if wBC is not None:
            uB = u_tiles[wBC]

        for t in range(SKT):
            if wA is not None and not V("noscores"):
                tsl = slice(t * P, (t + 1) * P)
                psr = psA.tile([P, STRIP], F32, tag="psA", name="psA")
                psp = psA.tile([P, STRIP], F32, tag="psA", name="psA")
                nc.tensor.matmul(psr[:], kcr[hA][:, tsl], qc[hA][:, sslA],
                                 start=True, stop=True)
                nc.tensor.matmul(psp[:], kcp[hA][:, tsl], qc[hA][:, sslA],
                                 start=True, stop=True)
                if not V("nosq"):
                    usl = uA[:, t, :]
                    nc.scalar.square(usl, psr[:])
                    nc.vector._custom_dve(SQADD, out=usl, in0=psp[:], in1=usl)
            if wDEF is not None:
                if not V("norowsum"):
                    nc.tensor.matmul(ps_rs[0:1, :], ones[:], attn[:, t, :],
                                     start=(t == 0), stop=(t == SKT - 1))
                if not V("noav"):
                    nc.tensor.matmul(ps_av[:], v2[hD][:, t, :], attn[:, t, :],
                                     start=(t == 0), stop=(t == SKT - 1))
            if wBC is not None and t % 4 == 3 and not V("nosqrtexp"):
                c = t // 4
                csl = uB[:, 4 * c:4 * (c + 1), :]
                nc.scalar.activation(csl, csl, AF.Sqrt)
                nc.vector._custom_dve(EXP8, out=csl, in0=csl,
                                      s0=EA, s1=EB, imm2=EC)

        if wDEF is not None:
            rrec = rrpool.tile([1, STRIP], F32, tag="rrec", name="rrec")
            if V("norecip"):
                nc.vector.memset(rrec[:], 1.0)
            else:
                nc.vector.reciprocal_approx_fast(rrec[:], ps_rs[0:1, :])
            rb = rbpool.tile([P, STRIP], F32, tag="rb", name="rb")
            if V("nobcast"):
                nc.vector.memset(rb[:], 1.0)
            else:
                nc.gpsimd.partition_broadcast(rb[:], rrec[:])
            lo, hi = slice(0, DK), slice(DK, P)
            if hD == 0:
                nc.vector.tensor_mul(xrT[lo, sslD], ps_av[lo, :], rb[lo, :])
                nc.vector.tensor_mul(xpT[hi, sslD], ps_av[hi, :], rb[hi, :])
            else:
                nc.vector.tensor_mul(xpT[lo, sslD], ps_av[lo, :], rb[lo, :])
                nc.vector.tensor_mul(xrT[hi, sslD], ps_av[hi, :], rb[hi, :])
            if hD == 1:
                stage_g(sD)

    # ---- emission: projections then software-pipelined attention --------
    for s in range(NSTRIP):
        k_proj(s)
    for s in range(NSTRIP):
        q_proj(s)
    for s in range(NSTRIP):
        v_proj(s)

    n = len(units)
    for slot in range(n + 2):
        slot_emit(units[slot] if slot < n else None,
                  units[slot - 1] if 1 <= slot <= n else None,
                  units[slot - 2] if slot >= 2 else None)


# ---------------------------------------------------------------------------
_CACHE = {}


def _get_nc(n_iter=1, variant=frozenset()):
    key = (n_iter, variant)
    if key not in _CACHE:
        _CACHE[key] = build(n_iter, variant)
    return _CACHE[key]


def make_in_maps(q_real, k_real, v_real, q_phase, k_phase, v_phase,
                 w_q, w_k, w_v, w_o):
    """Host-side shard + layout prep: per-core input dicts."""
    xt = {}
    for b in range(B):
        xt[("xqr", b)] = np.ascontiguousarray(q_real[b].T).astype(F16NP)
        xt[("xqp", b)] = np.ascontiguousarray(q_phase[b].T).astype(F16NP)
        xt[("xkr", b)] = np.ascontiguousarray(k_real[b].T).astype(F16NP)
        xt[("xkp", b)] = np.ascontiguousarray(k_phase[b].T).astype(F16NP)
        xt[("xvr", b)] = np.ascontiguousarray(v_real[b].T).astype(F16NP)
        xt[("xvp", b)] = np.ascontiguousarray(v_phase[b].T).astype(F16NP)
    wq16, wk16, wv16, wo16 = (w.astype(F16NP) for w in (w_q, w_k, w_v, w_o))
    in_maps = []
    for core in range(N_CORES):
        b, hg = divmod(core, HG)
        c0 = slice(hg * 2 * DK, hg * 2 * DK + DK)         # head h0 cols
        c1 = slice(hg * 2 * DK + DK, (hg + 1) * 2 * DK)   # head h1 cols
        def nsw(w):
            n = np.ascontiguousarray(np.concatenate([w[:, c0], w[:, c1]], 1))
            s = np.ascontiguousarray(np.concatenate([w[:, c1], w[:, c0]], 1))
            return n, s
        wqn, wqs = nsw(wq16)
        wkn, wks = nsw(wk16)
        wvn, wvs = nsw(wv16)
        wo_a = np.ascontiguousarray(np.concatenate([wo16[c0, :], wo16[c1, :]], 0))
        wo_b = np.ascontiguousarray(np.concatenate([wo16[c1, :], wo16[c0, :]], 0))
        in_maps.append({
            "xqr": xt[("xqr", b)], "xqp": xt[("xqp", b)],
            "xkr": xt[("xkr", b)], "xkp": xt[("xkp", b)],
            "xvr": xt[("xvr", b)], "xvp": xt[("xvp", b)],
            "wq_n": wqn, "wq_s": wqs,
            "wk_n": wkn, "wk_s": wks,
            "wv_n": wvn, "wv_s": wvs,
            "wo_a": wo_a, "wo_b": wo_b,
        })
    return in_maps


def gather_outputs(results):
    out_r = np.zeros((B, S, D), np.float32)
    out_p = np.zeros((B, S, D), np.float32)
    for core in range(N_CORES):
        b = core // HG
        out_r[b] += np.asarray(results[core]["o_r"], np.float32)
        out_p[b] += np.asarray(results[core]["o_p"], np.float32)
    return out_r, out_p


def _numpy_fallback(q_real, k_real, v_real, q_phase, k_phase, v_phase,
                    w_q, w_k, w_v, w_o, mask):
    def heads(x, w):
        y = x @ w
        return y.reshape(B, -1, H, DK).transpose(0, 2, 1, 3)
    qr, kr, vr = heads(q_real, w_q), heads(k_real, w_k), heads(v_real, w_v)
    qp, kp, vp = heads(q_phase, w_q), heads(k_phase, w_k), heads(v_phase, w_v)
    ar = np.einsum('bhqd,bhkd->bhqk', qr, kr) - np.einsum('bhqd,bhkd->bhqk', qp, kp)
    ap = np.einsum('bhqd,bhkd->bhqk', qr, kp) + np.einsum('bhqd,bhkd->bhqk', qp, kr)
    a = np.sqrt(ar * ar + ap * ap) / SCALE
    a = np.where(mask[:, None, :, :] == 0, np.float32(-1e9), a)
    a = a - a.max(axis=-1, keepdims=True)
    e = np.exp(a)
    a = e / e.sum(axis=-1, keepdims=True)
    xr = np.einsum('bhqk,bhkd->bhqd', a, vr).transpose(0, 2, 1, 3).reshape(B, -1, D)
    xp = np.einsum('bhqk,bhkd->bhqd', a, vp).transpose(0, 2, 1, 3).reshape(B, -1, D)
    return (xr @ w_o).astype(np.float32), (xp @ w_o).astype(np.float32)


def kernel(q_real, k_real, v_real, q_phase, k_phase, v_phase,
           w_q, w_k, w_v, w_o, mask):
    args = [np.asarray(a, np.float32) for a in
            (q_real, k_real, v_real, q_phase, k_phase, v_phase,
             w_q, w_k, w_v, w_o)]
    mask = np.asarray(mask)
    if not np.all(mask != 0):
        return _numpy_fallback(*args, mask)
    nc = _get_nc(1)
    in_maps = make_in_maps(*args)
    res = run_bass_kernel_spmd(nc, in_maps, core_ids=list(range(N_CORES)))
    return gather_outputs(res.results)


# revision 30
# speedup vs baseline: 1.3436x; 1.0180x over previous
"""Trainium2 Bass kernel for nn_MultiHeadAttention_65773129171319.

Complex-valued multi-head attention:
  attn = softmax(|Qc Kc^H| / sqrt(2 dk)) ; out = (attn @ Vr) Wo, (attn @ Vp) Wo

Sharding: 8 cores = 2 (batch) x 4 (head-groups of 2 heads).  Each core
computes its batch's full sequence for its 2 heads; the out-projection
partial sums (over head groups) are reduced on the host.

V2 design (all fp16 on device, fp32 PSUM):
  - Packed 2-head K/Q projections: one M=128 matmul per (plane, d-tile)
    computes both heads at once; plane pairs land in a single 2-bank PSUM
    tile, one ACT copy stages them to SBUF, and the GPSIMD (pool) engine
    repacks them into per-head stacked score operands:
      qc[0]=[Qr_h0;Qp_h0]  qc[1]=[Qp_h1;Qr_h1]
      kcr[0]=[Kr_h0;-Kp_h0] kcr[1]=[-Kp_h1;Kr_h1]
      kcp[0]=[Kp_h0;Kr_h0]  kcp[1]=[Kr_h1;Kp_h1]
    (The phase-plane matmuls use head-swapped weight copies so most of the
    repack copies are partition-offset-free.)
  - Scores per (strip, head): sT_r = kcr^T qc, sT_p = kcp^T qc as single
    C=128 matmuls producing transposed [sk, sq] tiles, two t-tiles per
    2-bank PSUM tile.  u = sT_r^2 + sT_p^2 via ACT Square + DVE SQADD.
  - m = sqrt(u) on ACT (the only table function -> zero table switches),
    attn = exp(m/SCALE) on DVE via custom EXP8 op:
      exp(m/s) ~= (((a*m + b)*m + c)^2)^2)^2   (max rel err ~5e-4)
  - Rowsums via ones-stationary matmul; reciprocal + partition broadcast;
    applied to the AV output (small side).
  - AV packed per head: stationary [vr_h|vp_h] (head1: [vp|vr]) gives
    M=128 AV matmuls; results scatter offset-free into xrT=[h0r;h1r] and
    xpT=[h1p;h0p]; out-projection uses wo_A (natural rows) for o_r and
    wo_B (head-swapped rows) for o_p.
  - Emission is software-pipelined over units w=(strip, head) in slots:
    scores(w) | sqrt+exp(w-1) | rowsum/AV/normalize(w-2), so PE, ACT and
    DVE always have a slot of ready work.
"""

import os
import sys

import numpy as np

try:
    import concourse.bass as bass
except ImportError:  # pragma: no cover
    sys.path.insert(0, "/opt/trn_rl_repo")
    import concourse.bass as bass

import concourse.mybir as mybir
import concourse.tile as tile
from concourse import bacc
from concourse.bass_utils import run_bass_kernel_spmd

B, S, D, H = 2, 2048, 512, 8
DK = D // H  # 64
SCALE = float((2 * DK) ** 0.5)  # sqrt(128)
P = 128
N_CORES = 8
HG = 4            # head groups (2 heads each)
DT = D // P       # 4 d-tiles for projection contraction
SKT = S // P      # 16 sk tiles
NSTRIP = 4        # sq strips of 512
STRIP = S // NSTRIP  # 512
NPAIR = SKT // 2  # t-pairs per (strip, head)

F32 = mybir.dt.float32
F16 = mybir.dt.float16
F16NP = np.float16

AF = mybir.ActivationFunctionType

# EXP8 poly coefficients: exp(m/SCALE) ~= (((EA*m+EB)*m+EC)^2^2)^2,
# fit on m in [0, 18] (empirical max |z| ~ 15.8); max rel err 5.3e-4.
EA = 6.734965764779986e-05
EB = 0.011003405951248851
EC = 1.0000654804195346


def register_custom_ops():
    """Register fused DVE ops (runtime extension of dve_ops.OPS)."""
    import concourse.dve_ops as dve_ops
    from concourse.dve_ops import DveOp
    from concourse.dve_spec import Spec, Src0, Src1, C0, C1, C2, sq, lower, _has_src1
    from concourse.dve_uop import DveOpSpec

    existing = {op.name: op for op in dve_ops.OPS}

    def mk(name, spec):
        if name in existing:
            return existing[name]
        row = max(dve_ops._SUB_OPCODE_FOR_NAME.values()) + 1
        assert row < 0x20, "no free DVE opcode rows"
        dve_ops._SUB_OPCODE_FOR_NAME[name] = row
        shas = {}
        for ver in ("v3", "v4"):
            s = DveOpSpec(name=name, opcode=row, uops=lower(spec, ver=ver),
                          rd1_en=_has_src1(spec))
            shas[ver] = s.sha(ver)
        op = DveOp(name, spec, subdim=False, uops_sha=shas)
        dve_ops.OPS.append(op)
        return op

    sqadd = mk("SQADD_ANT", Spec(
        body=sq(Src0) + Src1,
        reference=lambda in0, in1, s0, s1, imm2:
            in0.astype(np.float32) ** 2 + in1.astype(np.float32)))
    def _exp8_ref(in0, in1, s0, s1, imm2):
        p = (s0 * in0.astype(np.float32) + s1) * in0.astype(np.float32) + imm2
        return ((p ** 2) ** 2) ** 2

    exp8 = mk("EXP8_ANT", Spec(
        body=sq(sq(sq((Src0 * C0 + C1) * Src0 + C2))),
        reference=_exp8_ref))
    return sqadd, exp8


SQADD, EXP8 = register_custom_ops()


def build(n_iter: int = 1, variant: frozenset = frozenset()):
    """Build (and bacc-compile) the per-core SPMD program."""
    nc = bacc.Bacc("TRN2", target_bir_lowering=False, debug=False,
                   num_devices=N_CORES)

    dr = {}
    for name in ("xqr", "xqp", "xkr", "xkp", "xvr", "xvp"):
        dr[name] = nc.dram_tensor(name, [D, S], F16, kind="ExternalInput")
    for name in ("wq_n", "wq_s", "wk_n", "wk_s", "wv_n", "wv_s"):
        dr[name] = nc.dram_tensor(name, [D, 2 * DK], F16, kind="ExternalInput")
    dr["wo_a"] = nc.dram_tensor("wo_a", [2 * DK, D], F16, kind="ExternalInput")
    dr["wo_b"] = nc.dram_tensor("wo_b", [2 * DK, D], F16, kind="ExternalInput")
    dr["o_r"] = nc.dram_tensor("o_r", [S, D], F16, kind="ExternalOutput")
    dr["o_p"] = nc.dram_tensor("o_p", [S, D], F16, kind="ExternalOutput")

    with tile.TileContext(nc) as tc:
        _emit(tc, dr, n_iter, variant)
    nc.compile()
    return nc


def _emit(tc, dr, n_iter, variant=frozenset()):
    from contextlib import ExitStack

    ctx = ExitStack()
    with ctx:
        pools = dict(
            singles=ctx.enter_context(tc.tile_pool(name="singles", bufs=2)),
            xpool=ctx.enter_context(tc.tile_pool(name="xp", bufs=3)),
            ppool=ctx.enter_context(tc.tile_pool(name="pp", bufs=2)),
            upool=ctx.enter_context(tc.tile_pool(name="up", bufs=4)),
            rbpool=ctx.enter_context(tc.tile_pool(name="rb", bufs=2)),
            rrpool=ctx.enter_context(tc.tile_pool(name="rr", bufs=2)),
            opool=ctx.enter_context(tc.tile_pool(name="op", bufs=3)),
            psA=ctx.enter_context(tc.tile_pool(name="psA", bufs=6, space="PSUM")),
            psRS=ctx.enter_context(tc.tile_pool(name="psRS", bufs=1, space="PSUM")),
            psAV=ctx.enter_context(tc.tile_pool(name="psAV", bufs=1, space="PSUM")),
        )
        if n_iter > 1:
            # unroll x2 inside the hw loop so consecutive iterations use
            # alternating persistent buffers (singles bufs=2) and overlap.
            assert n_iter % 2 == 0, "n_iter must be even (or 1)"
            with tc.For_i(0, n_iter // 2, 1):
                _body(tc, dr, variant, **pools)
                _body(tc, dr, variant, **pools)
        else:
            _body(tc, dr, variant, **pools)


def _body(tc, dr, variant, singles, xpool, ppool, upool, rbpool, rrpool,
          opool, psA, psRS, psAV):
    nc = tc.nc
    V = lambda name: name in variant

    # ---- weights to SBUF -------------------------------------------------
    wsb = {}
    for name in ("wq_n", "wq_s", "wk_n", "wk_s", "wv_n", "wv_s"):
        t = singles.tile([P, DT, 2 * DK], F16, tag=f"w_{name}", name=f"w_{name}")
        nc.sync.dma_start(out=t[:], in_=dr[name].rearrange("(dt p) m -> p dt m", p=P))
        wsb[name] = t
    wo_a = singles.tile([P, D], F16, tag="w_wo_a", name="w_wo_a")
    nc.sync.dma_start(out=wo_a[:], in_=dr["wo_a"][:])
    wo_b = singles.tile([P, D], F16, tag="w_wo_b", name="w_wo_b")
    nc.sync.dma_start(out=wo_b[:], in_=dr["wo_b"][:])
    ones = singles.tile([P, 1], F16, tag="ones", name="ones")
    nc.vector.memset(ones[:], 1.0)

    # ---- persistent SBUF tensors ----------------------------------------
    kcr = [singles.tile([P, S], F16, tag=f"kcr{h}", name=f"kcr{h}") for h in range(2)]
    kcp = [singles.tile([P, S], F16, tag=f"kcp{h}", name=f"kcp{h}") for h in range(2)]
    qc = [singles.tile([P, S], F16, tag=f"qc{h}", name=f"qc{h}") for h in range(2)]
    v2 = [singles.tile([P, SKT, P], F16, tag=f"v2_{h}", name=f"v2_{h}") for h in range(2)]
    xrT = singles.tile([P, S], F16, tag="xrT", name="xrT")
    xpT = singles.tile([P, S], F16, tag="xpT", name="xpT")

    def _xdma(out, in_):
        if not V("nodma"):
            nc.sync.dma_start(out=out, in_=in_)

    def _xs(name, ssl):
        t = xpool.tile([P, DT, STRIP], F16, tag="xs", name="xs")
        _xdma(t[:], dr[name].rearrange("(dt p) s -> p dt s", p=P)[:, :, ssl])
        return t

    # ---- K projection + repack ------------------------------------------
    def k_proj(s):
        ssl = slice(s * STRIP, (s + 1) * STRIP)
        xtr = _xs("xkr", ssl)
        xtp = _xs("xkp", ssl)
        psKa = psA.tile([P, STRIP], F32, tag="psA", name="psA")
        psKb = psA.tile([P, STRIP], F32, tag="psA", name="psA")
        if not V("noproj"):
            for dt in range(DT):
                st, sp = (dt == 0), (dt == DT - 1)
                nc.tensor.matmul(psKa[:], wsb["wk_n"][:, dt, :],
                                 xtr[:, dt, :], start=st, stop=sp)
                nc.tensor.matmul(psKb[:], wsb["wk_s"][:, dt, :],
                                 xtp[:, dt, :], start=st, stop=sp)
        pl = ppool.tile([P, 2, STRIP], F16, tag="pl", name="pl")
        nc.scalar.copy(pl[:, 0, :], psKa[:])
        nc.scalar.copy(pl[:, 1, :], psKb[:])
        lo, hi = slice(0, DK), slice(DK, P)
        dv = nc.gpsimd
        # plane 0 = [Kr_h0; Kr_h1], plane 1 = [Kp_h1; Kp_h0] (partition halves)
        dv.tensor_copy(kcr[0][lo, ssl], pl[lo, 0, :])
        dv.tensor_scalar_mul(kcr[0][hi, ssl], pl[hi, 1, :], -1.0)
        dv.tensor_scalar_mul(kcr[1][lo, ssl], pl[lo, 1, :], -1.0)
        dv.tensor_copy(kcr[1][hi, ssl], pl[hi, 0, :])
        dv.tensor_copy(kcp[0][lo, ssl], pl[hi, 1, :])
        dv.tensor_copy(kcp[0][hi, ssl], pl[lo, 0, :])
        dv.tensor_copy(kcp[1][lo, ssl], pl[hi, 0, :])
        dv.tensor_copy(kcp[1][hi, ssl], pl[lo, 1, :])

    # ---- Q projection + repack ------------------------------------------
    def q_proj(s):
        ssl = slice(s * STRIP, (s + 1) * STRIP)
        xtr = _xs("xqr", ssl)
        xtp = _xs("xqp", ssl)
        psQa = psA.tile([P, STRIP], F32, tag="psA", name="psA")
        psQb = psA.tile([P, STRIP], F32, tag="psA", name="psA")
        if not V("noproj"):
            for dt in range(DT):
                st, sp = (dt == 0), (dt == DT - 1)
                nc.tensor.matmul(psQa[:], wsb["wq_n"][:, dt, :],
                                 xtr[:, dt, :], start=st, stop=sp)
                nc.tensor.matmul(psQb[:], wsb["wq_s"][:, dt, :],
                                 xtp[:, dt, :], start=st, stop=sp)
        pl = ppool.tile([P, 2, STRIP], F16, tag="pl", name="pl")
        nc.scalar.copy(pl[:, 0, :], psQa[:])
        nc.scalar.copy(pl[:, 1, :], psQb[:])
        lo, hi = slice(0, DK), slice(DK, P)
        dv = nc.gpsimd
        # plane 0 = [Qr_h0; Qr_h1], plane 1 = [Qp_h1; Qp_h0]
        dv.tensor_copy(qc[0][lo, ssl], pl[lo, 0, :])
        dv.tensor_copy(qc[0][hi, ssl], pl[hi, 1, :])
        dv.tensor_copy(qc[1][lo, ssl], pl[lo, 1, :])
        dv.tensor_copy(qc[1][hi, ssl], pl[hi, 0, :])

    # ---- V projection ----------------------------------------------------
    def v_proj(s):
        xvr_t = _xs("xvr", slice(s * STRIP, (s + 1) * STRIP))
        xvp_t = _xs("xvp", slice(s * STRIP, (s + 1) * STRIP))
        for tt in range(STRIP // P):
            t = s * (STRIP // P) + tt
            tpsl = slice(tt * P, (tt + 1) * P)
            psVa = psA.tile([P, STRIP], F32, tag="psA", name="psA")
            psVb = psA.tile([P, STRIP], F32, tag="psA", name="psA")
            if not V("noproj"):
                for dt in range(DT):
                    st, sp = (dt == 0), (dt == DT - 1)
                    nc.tensor.matmul(psVa[:, 0:2 * DK], xvr_t[:, dt, tpsl],
                                     wsb["wv_n"][:, dt, :], start=st, stop=sp)
                    nc.tensor.matmul(psVb[:, 0:2 * DK], xvp_t[:, dt, tpsl],
                                     wsb["wv_s"][:, dt, :], start=st, stop=sp)
            # psVa = [vr_h0 | vr_h1], psVb = [vp_h1 | vp_h0]
            nc.scalar.copy(v2[0][:, t, 0:DK], psVa[:, 0:DK])
            nc.vector.tensor_copy(v2[0][:, t, DK:2 * DK], psVb[:, DK:2 * DK])
            nc.scalar.copy(v2[1][:, t, 0:DK], psVb[:, 0:DK])
            nc.vector.tensor_copy(v2[1][:, t, DK:2 * DK], psVa[:, DK:2 * DK])

    # ---- attention stages -----------------------------------------------
    units = [(s, h) for s in range(NSTRIP) for h in range(2)]
    u_tiles = {}

    def stage_g(s):
        if V("noout"):
            return
        for q in range(STRIP // P):
            qsl = slice((s * (STRIP // P) + q) * P, (s * (STRIP // P) + q + 1) * P)
            for xT, wo, out in ((xrT, wo_a, dr["o_r"]), (xpT, wo_b, dr["o_p"])):
                ps_o = psA.tile([P, D], F32, tag="psA", name="psA")
                nc.tensor.matmul(ps_o[:], xT[:, qsl], wo[:], start=True, stop=True)
                osb = opool.tile([P, D], F16, tag="osb", name="osb")
                nc.scalar.copy(osb[:], ps_o[:])
                # output DMA on the pool queue so the SP queue stays free to
                # prefetch the next iteration's inputs.
                if not V("nodma"):
                    nc.gpsimd.dma_start(out=out[qsl, :], in_=osb[:])

    def slot_emit(wA, wBC, wDEF):
        """One pipeline slot, interleaved per sk-tile t so no engine sees a
        long burst of another stage:
          wA:   scores + square + sqadd        (PE + ACT + DVE)
          wBC:  sqrt + exp chunks of wA's predecessor (ACT + DVE)
          wDEF: rowsum + AV + normalize of the unit before that (PE + DVE)
        """
        if wA is not None:
            sA, hA = wA
            sslA = slice(sA * STRIP, (sA + 1) * STRIP)
            uA = upool.tile([P, SKT, STRIP], F16, tag="u", name="u")
            u_tiles[wA] = uA
            if V("nosq"):
                nc.vector.memset(uA[:], 1.0)
        if wDEF is not None:
            sD, hD = wDEF
            sslD = slice(sD * STRIP, (sD + 1) * STRIP)
            attn = u_tiles.pop(wDEF)
            ps_rs = psRS.tile([1, STRIP], F32, tag="rs", name="rs")
            ps_av = psAV.tile([P, STRIP], F32, tag="av", name="av")
        if wBC is not None:
            uB = u_tiles[wBC]

        for t in range(SKT):
            if wA is not None and not V("noscores"):
                tsl = slice(t * P, (t + 1) * P)
                psr = psA.tile([P, STRIP], F32, tag="psA", name="psA")
                psp = psA.tile([P, STRIP], F32, tag="psA", name="psA")
                nc.tensor.matmul(psr[:], kcr[hA][:, tsl], qc[hA][:, sslA],
                                 start=True, stop=True)
                nc.tensor.matmul(psp[:], kcp[hA][:, tsl], qc[hA][:, sslA],
                                 start=True, stop=True)
                if not V("nosq"):
                    usl = uA[:, t, :]
                    nc.scalar.square(usl, psr[:])
                    nc.vector._custom_dve(SQADD, out=usl, in0=psp[:], in1=usl)
            if wDEF is not None:
                if not V("norowsum"):
                    nc.tensor.matmul(ps_rs[0:1, :], ones[:], attn[:, t, :],
                                     start=(t == 0), stop=(t == SKT - 1))
                if not V("noav"):
                    nc.tensor.matmul(ps_av[:], v2[hD][:, t, :], attn[:, t, :],
                                     start=(t == 0), stop=(t == SKT - 1))
            if wBC is not None and t % 4 == 1 and not V("nosqrtexp"):
                c = t // 4
                csl = uB[:, 4 * c:4 * (c + 1), :]
                nc.scalar.activation(csl, csl, AF.Sqrt)
            if wBC is not None and t % 4 == 3 and not V("nosqrtexp"):
                c = t // 4
                csl = uB[:, 4 * c:4 * (c + 1), :]
                nc.vector._custom_dve(EXP8, out=csl, in0=csl,
                                      s0=EA, s1=EB, imm2=EC)

        if wDEF is not None:
            rrec = rrpool.tile([1, STRIP], F32, tag="rrec", name="rrec")
            if V("norecip"):
                nc.vector.memset(rrec[:], 1.0)
            else:
                nc.vector.reciprocal_approx_fast(rrec[:], ps_rs[0:1, :])
            rb = rbpool.tile([P, STRIP], F32, tag="rb", name="rb")
            if V("nobcast"):
                nc.vector.memset(rb[:], 1.0)
            else:
                nc.gpsimd.partition_broadcast(rb[:], rrec[:])
            lo, hi = slice(0, DK), slice(DK, P)
            if hD == 0:
                nc.vector.tensor_mul(xrT[lo, sslD], ps_av[lo, :], rb[lo, :])
                nc.vector.tensor_mul(xpT[hi, sslD], ps_av[hi, :], rb[hi, :])
            else:
                nc.vector.tensor_mul(xpT[lo, sslD], ps_av[lo, :], rb[lo, :])
                nc.vector.tensor_mul(xrT[hi, sslD], ps_av[hi, :], rb[hi, :])
            if hD == 1:
                stage_g(sD)

    # ---- emission: projections then software-pipelined attention --------
    for s in range(NSTRIP):
        k_proj(s)
    for s in range(NSTRIP):
        q_proj(s)
    for s in range(NSTRIP):
        v_proj(s)

    n = len(units)
    for slot in range(n + 2):
        slot_emit(units[slot] if slot < n else None,
                  units[slot - 1] if 1 <= slot <= n else None,
                  units[slot - 2] if slot >= 2 else None)


# ---------------------------------------------------------------------------
_CACHE = {}


def _get_nc(n_iter=1, variant=frozenset()):
    key = (n_iter, variant)
    if key not in _CACHE:
        _CACHE[key] = build(n_iter, variant)
    return _CACHE[key]


def make_in_maps(q_real, k_real, v_real, q_phase, k_phase, v_phase,
                 w_q, w_k, w_v, w_o):
    """Host-side shard + layout prep: per-core input dicts."""
    xt = {}
    for b in range(B):
        xt[("xqr", b)] = np.ascontiguousarray(q_real[b].T).astype(F16NP)
        xt[("xqp", b)] = np.ascontiguousarray(q_phase[b].T).astype(F16NP)
        xt[("xkr", b)] = np.ascontiguousarray(k_real[b].T).astype(F16NP)
        xt[("xkp", b)] = np.ascontiguousarray(k_phase[b].T).astype(F16NP)
        xt[("xvr", b)] = np.ascontiguousarray(v_real[b].T).astype(F16NP)
        xt[("xvp", b)] = np.ascontiguousarray(v_phase[b].T).astype(F16NP)
    wq16, wk16, wv16, wo16 = (w.astype(F16NP) for w in (w_q, w_k, w_v, w_o))
    in_maps = []
    for core in range(N_CORES):
        b, hg = divmod(core, HG)
        c0 = slice(hg * 2 * DK, hg * 2 * DK + DK)         # head h0 cols
        c1 = slice(hg * 2 * DK + DK, (hg + 1) * 2 * DK)   # head h1 cols
        def nsw(w):
            n = np.ascontiguousarray(np.concatenate([w[:, c0], w[:, c1]], 1))
            s = np.ascontiguousarray(np.concatenate([w[:, c1], w[:, c0]], 1))
            return n, s
        wqn, wqs = nsw(wq16)
        wkn, wks = nsw(wk16)
        wvn, wvs = nsw(wv16)
        wo_a = np.ascontiguousarray(np.concatenate([wo16[c0, :], wo16[c1, :]], 0))
        wo_b = np.ascontiguousarray(np.concatenate([wo16[c1, :], wo16[c0, :]], 0))
        in_maps.append({
            "xqr": xt[("xqr", b)], "xqp": xt[("xqp", b)],
            "xkr": xt[("xkr", b)], "xkp": xt[("xkp", b)],
            "xvr": xt[("xvr", b)], "xvp": xt[("xvp", b)],
            "wq_n": wqn, "wq_s": wqs,
            "wk_n": wkn, "wk_s": wks,
            "wv_n": wvn, "wv_s": wvs,
            "wo_a": wo_a, "wo_b": wo_b,
        })
    return in_maps


def gather_outputs(results):
    out_r = np.zeros((B, S, D), np.float32)
    out_p = np.zeros((B, S, D), np.float32)
    for core in range(N_CORES):
        b = core // HG
        out_r[b] += np.asarray(results[core]["o_r"], np.float32)
        out_p[b] += np.asarray(results[core]["o_p"], np.float32)
    return out_r, out_p


def _numpy_fallback(q_real, k_real, v_real, q_phase, k_phase, v_phase,
                    w_q, w_k, w_v, w_o, mask):
    def heads(x, w):
        y = x @ w
        return y.reshape(B, -1, H, DK).transpose(0, 2, 1, 3)
    qr, kr, vr = heads(q_real, w_q), heads(k_real, w_k), heads(v_real, w_v)
    qp, kp, vp = heads(q_phase, w_q), heads(k_phase, w_k), heads(v_phase, w_v)
    ar = np.einsum('bhqd,bhkd->bhqk', qr, kr) - np.einsum('bhqd,bhkd->bhqk', qp, kp)
    ap = np.einsum('bhqd,bhkd->bhqk', qr, kp) + np.einsum('bhqd,bhkd->bhqk', qp, kr)
    a = np.sqrt(ar * ar + ap * ap) / SCALE
    a = np.where(mask[:, None, :, :] == 0, np.float32(-1e9), a)
    a = a - a.max(axis=-1, keepdims=True)
    e = np.exp(a)
    a = e / e.sum(axis=-1, keepdims=True)
    xr = np.einsum('bhqk,bhkd->bhqd', a, vr).transpose(0, 2, 1, 3).reshape(B, -1, D)
    xp = np.einsum('bhqk,bhkd->bhqd', a, vp).transpose(0, 2, 1, 3).reshape(B, -1, D)
    return (xr @ w_o).astype(np.float32), (xp @ w_o).astype(np.float32)


def kernel(q_real, k_real, v_real, q_phase, k_phase, v_phase,
           w_q, w_k, w_v, w_o, mask):
    args = [np.asarray(a, np.float32) for a in
            (q_real, k_real, v_real, q_phase, k_phase, v_phase,
             w_q, w_k, w_v, w_o)]
    mask = np.asarray(mask)
    if not np.all(mask != 0):
        return _numpy_fallback(*args, mask)
    nc = _get_nc(1)
    in_maps = make_in_maps(*args)
    res = run_bass_kernel_spmd(nc, in_maps, core_ids=list(range(N_CORES)))
    return gather_outputs(res.results)
